# revision 55
# baseline (speedup 1.0000x reference)
"""Trainium2 Bass kernel for nn_EndToEndRPModel.

Pipeline per sample: conv1d stack (8ch,T=512 -> 6ch) -> pairwise-distance
soft recurrence plot (512x512) -> bilinear resize to 64x64 (exact 2x2 mean
of a strided 128x128 subgrid since scale=8) -> min-max norm -> small CNN ->
FC head -> scalar.

Sharding: pure data parallel, 8 samples per core on 8 cores.

Key implementation notes:
 - all heavy matmuls run in fp16 (1 cyc/col); d2 = sq_i + sq_j - 2*gram via
   ONE augmented fp16 matmul per 128-row tile, 4 samples packed into
   disjoint PE row quadrants via tile_position.
 - d2 diagonal forced to dist=1e-3 with gpsimd.affine_select restricted to
   the 16-col diagonal window of each row tile.
 - bilinear(512->64) == 0.25 * 2x2-sum over rows/cols {8j+3, 8j+4}; row
   selection+0.25 folded into a pooling matmul, col selection into the exp.
 - conv1d-2 / CNN L2 / CNN L3 run as tap-pair matmuls (K=128: two
   column-shifted copies of the input stacked in the partition dim),
   plus one single-tap matmul for the odd tap column.
 - CNN L1 runs as K=36 matmuls with all 9 taps baked into 9 dy/dx-shifted
   partition bands of the im2col tile (built with cheap vector copies).
 - sigma chain batched per group (one reduce + 2 tiny matmuls for 4
   samples).
 - narrow constants ship transposed (few wide DMA descriptors) and are
   transposed back on the PE at startup; identities/memsets are emitted
   before any Pool-queue DMA so nothing blocks them.
 - all BN affines folded into the Gelu activation's per-partition
   scale/bias; avgpool's 0.25 folded into the FC1 weights.
"""
import sys

sys.path.insert(0, "/opt/trn_rl_repo")

import numpy as np

import concourse.bacc as bacc
import concourse.tile as tile
from concourse import mybir
from concourse.bass_utils import run_bass_kernel_spmd
from concourse.masks import make_identity

f32 = mybir.dt.float32
f32r = mybir.dt.float32r
f16 = mybir.dt.float16
AF = mybir.ActivationFunctionType
ALU = mybir.AluOpType

N_CORES = 8
SPC = 8          # samples per core
T = 512
BN_KAPPA = 1.0 / np.sqrt(1.0 + 1e-5)


# ---------------------------------------------------------------- host-side
def _pack_consts(inp):
    """Pack all weights into the exact SBUF layouts the kernel uses."""
    c16 = {}
    c32 = {}
    w1 = inp["w1"]; w2 = inp["w2"]; w3 = inp["w3"]

    # conv1d-1 im2col weights: rows 16k + 8s2 + ch, cols 32s2 + o
    # shipped transposed [64, 112] and PE-transposed on chip.
    w1imT = np.zeros((112, 64), np.float32)
    for k in range(7):
        for s2 in range(2):
            w1imT[16 * k + 8 * s2:16 * k + 8 * s2 + 8, 32 * s2:32 * s2 + 32] = \
                w1[:, :, k].T
    c16["w1imTT"] = np.ascontiguousarray(w1imT.T)

    # conv1d-2 tap-pair weights: rounds (0,1), (2,3) are [128, 128]
    # (rows 64b + 32s2 + ch for band b in {tap k, tap k+1}); round 4 is
    # [64, 128] single-tap.  Shipped as one [128, 3, 128] tensor.
    w2p = np.zeros((128, 3, 128), np.float32)
    for rnd, k0 in enumerate((0, 2)):
        for b in range(2):
            for s2 in range(2):
                w2p[64 * b + 32 * s2:64 * b + 32 * s2 + 32, rnd,
                    64 * s2:64 * s2 + 64] = w2[:, :, k0 + b].T
    for s2 in range(2):
        w2p[32 * s2:32 * s2 + 32, 2, 64 * s2:64 * s2 + 64] = w2[:, :, 4].T
    c16["w2p"] = w2p

    # conv1d-3 taps: (128, 3, 12): rows 64s2+ch, cols 6s2+d
    # shipped transposed [36, 128] (rows 12k + 6s2 + d) and PE-transposed,
    # then scattered into even/odd-pair lhsT tiles on chip (z output lands
    # at 32-aligned psum rows 32sg+d so DVE can copy it).
    w3T = np.zeros((128, 3, 12), np.float32)
    for k in range(3):
        for s2 in range(2):
            w3T[64 * s2:64 * s2 + 64, k, 6 * s2:6 * s2 + 6] = w3[:, :, k].T
    c16["w3TT"] = np.ascontiguousarray(w3T.reshape(128, 36).T)

    # sq selector: rows 32sg + d -> col s2 (even/odd pair via 64-row halves)
    sqsel = np.zeros((128, 2), np.float32)
    for h in range(2):
        for s2 in range(2):
            sqsel[64 * h + 32 * s2:64 * h + 32 * s2 + 6, s2] = 1.0
    c32["sqselT"] = np.ascontiguousarray(sqsel.T)

    # pooling matrix for rp row-pairs: p025[p, r, j] = 0.25 if 128r+p in {8j+3, 8j+4}
    p025 = np.zeros((128, 4, 64), np.float32)
    for r in range(4):
        for p in range(128):
            i = 128 * r + p
            if i % 8 in (3, 4):
                j = (i - 3) // 8 if i % 8 == 3 else (i - 4) // 8
                if 0 <= j < 64:
                    p025[p, r, j] = 0.25
    c16["p025"] = p025

    # min-max combiner: mnmx8 rows = [mx0..mx3, -mn0..-mn3]
    m8 = np.zeros((8, 8), np.float32)
    for s in range(4):
        m8[s, s] = m8[4 + s, s] = 1.0    # den_s = mx_s + (-mn_s)
        m8[4 + s, 4 + s] = 1.0           # negmn_s
    c32["m8sel"] = m8

    # 2D conv weights
    c1 = inp["c1"]; c2 = inp["c2"]; c3 = inp["c3"]; c4 = inp["c4"]
    # L1: K=36 im2col, rows 12dx + 4dy + s, cols 32s + o
    c1imT = np.zeros((36, 128), np.float32)
    for s in range(4):
        for dy in range(3):
            for dx in range(3):
                c1imT[12 * dx + 4 * dy + s, 32 * s:32 * s + 32] = c1[:, 0, dy, dx]
    c16["c1imT"] = c1imT

    # L2 tap-pair weights: bands [s0, s1, s0<<1col, s1<<1col] x 32ch
    w2p2 = np.zeros((128, 3, 128), np.float32)
    w2s2 = np.zeros((64, 3, 128), np.float32)
    for dy in range(3):
        for s2 in range(2):
            w2p2[32 * s2:32 * s2 + 32, dy, 64 * s2:64 * s2 + 64] = \
                c2[:, :, dy, 0].T
            w2p2[64 + 32 * s2:64 + 32 * s2 + 32, dy, 64 * s2:64 * s2 + 64] = \
                c2[:, :, dy, 1].T
            w2s2[32 * s2:32 * s2 + 32, dy, 64 * s2:64 * s2 + 64] = \
                c2[:, :, dy, 2].T
    c16["w2p2"] = w2p2
    c16["w2s2"] = w2s2

    # L3 tap-pair weights: bands [64ch, 64ch<<1col]
    w3p = np.zeros((128, 3, 128), np.float32)
    w3s = np.zeros((64, 3, 128), np.float32)
    for dy in range(3):
        w3p[0:64, dy, :] = c3[:, :, dy, 0].T
        w3p[64:128, dy, :] = c3[:, :, dy, 1].T
        w3s[:, dy, :] = c3[:, :, dy, 2].T
    c16["w3p"] = w3p
    c16["w3s"] = w3s

    cw4T = np.zeros((128, 9, 128), np.float32)
    for t in range(9):
        dy, dx = t // 3, t % 3
        cw4T[:, t, :] = c4[:, :, dy, dx].T
    c16["cw4T"] = cw4T

    # FC1 weights: (128, 16, 256), 0.25 avgpool folded in
    fc1_w = np.asarray(inp["fc1_w"], np.float32)        # (256, 2048)
    c16["fc1wT"] = 0.25 * np.ascontiguousarray(
        fc1_w.reshape(256, 128, 16).transpose(1, 2, 0))
    c16["fc1brow"] = inp["fc1_b"].reshape(1, 256).astype(np.float32)
    c32["fc2wb"] = np.broadcast_to(
        inp["fc2_w"].reshape(1, 256), (8, 256)).astype(np.float32).copy()
    c32["fc2bias"] = np.full(
        (8, 1), float(np.asarray(inp["fc2_b"]).reshape(-1)[0]), np.float32)

    # BN scale/bias vectors, one [16, 128] f32 pack shipped transposed.
    def rep(v, reps):
        return np.tile(np.asarray(v, np.float32), reps)
    bnT = np.zeros((16, 128), np.float32)
    bnT[0] = rep(inp["g1"] * BN_KAPPA, 4); bnT[1] = rep(inp["b1"], 4)
    bnT[2] = rep(inp["g2"] * BN_KAPPA, 2); bnT[3] = rep(inp["b2"], 2)
    bnT[4] = rep(inp["cg1"] * BN_KAPPA, 4); bnT[5] = rep(inp["cb1"], 4)
    bnT[6] = rep(inp["cg2"] * BN_KAPPA, 2); bnT[7] = rep(inp["cb2"], 2)
    bnT[8] = inp["cg3"] * BN_KAPPA; bnT[9] = inp["cb3"]
    bnT[10] = inp["cg4"] * BN_KAPPA; bnT[11] = inp["cb4"]
    c32["bnT"] = bnT

    out = {k: np.ascontiguousarray(v, np.float16) for k, v in c16.items()}
    out.update({k: np.ascontiguousarray(v, np.float32) for k, v in c32.items()})
    return out


# ------------------------------------------------------------- bass program
_C16_SHAPES = {
    "w1imTT": (64, 112), "w2p": (128, 3, 128), "w3TT": (36, 128),
    "p025": (128, 4, 64), "c1imT": (36, 128), "w2p2": (128, 3, 128),
    "w2s2": (64, 3, 128), "w3p": (128, 3, 128), "w3s": (64, 3, 128),
    "cw4T": (128, 9, 128), "fc1wT": (128, 16, 256), "fc1brow": (1, 256),
}
_C32_SHAPES = {
    "sqselT": (2, 128), "m8sel": (8, 8), "fc2wb": (8, 256), "fc2bias": (8, 1),
    "bnT": (16, 128),
}

BN_COL = {"bn1s": 0, "bn1b": 1, "bn2s": 2, "bn2b": 3, "cbn1s": 4, "cbn1b": 5,
          "cbn2s": 6, "cbn2b": 7, "cbn3s": 8, "cbn3b": 9, "cbn4s": 10,
          "cbn4b": 11}


def build_program(debug=False):
    nc = bacc.Bacc("TRN2", target_bir_lowering=False, debug=False,
                   num_devices=N_CORES)
    xim = nc.dram_tensor("xim", [112, 4, T], f16, kind="ExternalInput").ap()
    dram = {n: nc.dram_tensor(n, list(s), f16, kind="ExternalInput").ap()
            for n, s in _C16_SHAPES.items()}
    dram.update({n: nc.dram_tensor(n, list(s), f32, kind="ExternalInput").ap()
                 for n, s in _C32_SHAPES.items()})
    out = nc.dram_tensor("out", [SPC, 1], f32, kind="ExternalOutput").ap()

    with tile.TileContext(nc) as tc:
        _emit(tc, nc, xim, dram, out)
    nc.compile()
    return nc


def _emit(tc, nc, xim, dram, out):
    from contextlib import ExitStack
    ctx = ExitStack()
    with ctx:
        cpool = ctx.enter_context(tc.tile_pool(name="consts", bufs=1))
        sing = ctx.enter_context(tc.tile_pool(name="sing", bufs=1))
        c1p = ctx.enter_context(tc.tile_pool(name="conv1", bufs=3))
        dstp = ctx.enter_context(tc.tile_pool(name="dist", bufs=3))
        pairp = ctx.enter_context(tc.tile_pool(name="pairs", bufs=2))
        ecolp = ctx.enter_context(tc.tile_pool(name="ecols", bufs=1))
        grpp = ctx.enter_context(tc.tile_pool(name="grp", bufs=1))
        l1p = ctx.enter_context(tc.tile_pool(name="lcnn", bufs=1))
        pbig = ctx.enter_context(tc.tile_pool(name="pbig", bufs=6, space="PSUM"))
        prp = ctx.enter_context(tc.tile_pool(name="prp", bufs=1, space="PSUM"))
        psml = ctx.enter_context(tc.tile_pool(name="psml", bufs=1, space="PSUM"))

        # ------------- persistent tiles (allocated before anything runs)
        zaug_m = [grpp.tile([128, T], f16, tag=f"zaug_m{g}", name=f"zaug_m{g}")
                  for g in range(2)]
        zaug_s = [grpp.tile([128, T], f16, tag=f"zaug_s{g}", name=f"zaug_s{g}")
                  for g in range(2)]
        # 2 extra cols so the (dy=2, dx=2) L1 im2col band read stays in range
        xpgrps = [grpp.tile([4, 66 * 66 + 2], f16, tag=f"xpg{g}",
                            name=f"xpg{g}") for g in range(2)]
        xl2b = {(g, q): l1p.tile([128, 34 * 34], f16, tag=f"xl2b_{g}_{q}",
                                 name=f"xl2b_{g}_{q}")
                for g in range(2) for q in range(2)}
        xl3b = {(g, q, s2): l1p.tile([128, 18 * 18], f16,
                                     tag=f"xl3b_{g}_{q}_{s2}",
                                     name=f"xl3b_{g}_{q}_{s2}")
                for g in range(2) for q in range(2) for s2 in range(2)}
        zsq = sing.tile([128, T], f16)
        sqr_sb = sing.tile([128, T], f16)     # pair p sq rows at 32p, 32p+1
        rs = sing.tile([128, 8, 4], f32)       # sqrt row-sums per (s, r)
        rrt = sing.tile([128, 8], f32)
        nrs = sing.tile([128, 8], f32)         # -1/sigma broadcast per sample
        fcin = sing.tile([128, 128], f16)
        fch = sing.tile([8, 256], f32)

        # ------------- setup on the (otherwise idle) gpsimd queue so the
        # vector queue stays free for the startup const copies.  Order:
        # identities (gate the PE transposes), then tiles needed by conv1d,
        # then the late-phase tiles.  zaug ones-rows are filled as [8, T]
        # 32-aligned strips: value!=0 memsets are ~10x slower than 0.0 and
        # cost scales with rows, and later z/-2z/sq writes overwrite 7 of 8.
        ident = cpool.tile([128, 128], f32)
        make_identity(nc, ident)
        identh = cpool.tile([64, 64], f16)
        make_identity(nc, identh)
        ones128x1 = cpool.tile([128, 1], f32)
        nc.gpsimd.memset(ones128x1, 1.0)
        ones1x128 = cpool.tile([1, 128], f32)
        nc.gpsimd.memset(ones1x128, 1.0)
        # conv1d-3 lhsT (cols 0:6 / 32:38 so both samples' z lands at
        # 32-aligned psum rows for every pair)
        w3T38 = cpool.tile([128, 3, 38], f16, name="w3T38")
        nc.gpsimd.memset(w3T38, 0.0)
        for g in range(2):
            for sg in range(4):
                nc.gpsimd.memset(zaug_m[g][32 * sg:32 * sg + 8, :], 1.0)
                nc.gpsimd.memset(zaug_s[g][32 * sg:32 * sg + 8, :], 1.0)
        nc.gpsimd.memset(zsq, 0.0)
        for g in range(2):
            nc.gpsimd.memset(xpgrps[g], 0.0)
        for t in xl2b.values():
            nc.gpsimd.memset(t, 0.0)
        for t in xl3b.values():
            nc.gpsimd.memset(t, 0.0)
        l4ins = [l1p.tile([128, 400], f16, tag=f"l4in{g}", name=f"l4in{g}")
                 for g in range(2)]
        nc.gpsimd.memset(l4ins[0], 0.0)
        nc.gpsimd.memset(l4ins[1], 0.0)

        # ---------------- constants into SBUF (sync + scalar queues)
        csb = {}

        def ctile(n):
            shape = _C16_SHAPES.get(n) or _C32_SHAPES[n]
            t = cpool.tile(list(shape), f16 if n in _C16_SHAPES else f32,
                           name="c_" + n, tag="c_" + n)
            csb[n] = t
            return t

        # Stage 0: first-matmul critical
        t_w1 = ctile("w1imTT")
        nc.sync.dma_start(out=t_w1[0:32], in_=dram["w1imTT"][0:32])
        nc.scalar.dma_start(out=t_w1[32:64], in_=dram["w1imTT"][32:64])
        imall = c1p.tile([112, 4, T], f16, tag="imall", bufs=1, name="imall")
        nc.sync.dma_start(out=imall[0:56, 0], in_=xim[0:56, 0])
        nc.scalar.dma_start(out=imall[56:112, 0], in_=xim[56:112, 0])

        # Stage 1: phase-1 weights + remaining input pairs.  The Act queue
        # stays on a DMA diet (gelu dispatch queues behind its DMAs), so
        # pairs 1-3 go fully on sync.
        t_bnT = ctile("bnT")
        nc.scalar.dma_start(out=t_bnT, in_=dram["bnT"])
        t_w2p = ctile("w2p")
        nc.sync.dma_start(out=t_w2p[0:64], in_=dram["w2p"][0:64])
        nc.scalar.dma_start(out=t_w2p[64:128], in_=dram["w2p"][64:128])
        t_w3 = ctile("w3TT")
        nc.sync.dma_start(out=t_w3, in_=dram["w3TT"])
        t_sq = ctile("sqselT")
        nc.scalar.dma_start(out=t_sq, in_=dram["sqselT"])
        for p in range(1, 4):
            nc.sync.dma_start(out=imall[0:56, p], in_=xim[0:56, p])
            nc.sync.dma_start(out=imall[56:112, p], in_=xim[56:112, p])
        QS = [nc.sync, nc.scalar]

        onesK1M8 = cpool.tile([1, 8], f16)
        nc.gpsimd.memset(onesK1M8, 1.0)

        # --------- on-chip transposes of narrow consts (also warms the PE)
        ps_t1 = psml.tile([112, 64], f16, tag="ps")
        nc.tensor.matmul(ps_t1, csb["w1imTT"], identh, is_transpose=True)
        w1imT = cpool.tile([112, 64], f16, name="w1imT")
        nc.vector.tensor_copy(out=w1imT, in_=ps_t1)

        ps_t2 = psml.tile([128, 36], f16, tag="ps")
        nc.tensor.matmul(ps_t2, csb["w3TT"], identh[0:36, 0:36],
                         is_transpose=True)
        w3Tsb = cpool.tile([128, 36], f16, name="w3Tsb")
        nc.vector.tensor_copy(out=w3Tsb, in_=ps_t2)
        w3v = w3Tsb.rearrange("p (k sd) -> p k sd", sd=12)
        for k in range(3):
            nc.vector.tensor_copy(out=w3T38[:, k, 0:6], in_=w3v[:, k, 0:6])
            nc.vector.tensor_copy(out=w3T38[:, k, 32:38], in_=w3v[:, k, 6:12])

        ps_t3 = psml.tile([128, 16], f32, tag="ps")
        nc.tensor.matmul(ps_t3, csb["bnT"], ident[0:16, 0:16],
                         is_transpose=True)
        bnsb = cpool.tile([128, 16], f32, name="bnsb")
        nc.vector.tensor_copy(out=bnsb, in_=ps_t3)

        ps_t4 = psml.tile([128, 2], f32, tag="ps")
        nc.tensor.matmul(ps_t4, csb["sqselT"], ident[0:2, 0:2],
                         is_transpose=True)
        sqsel = cpool.tile([128, 2], f16, name="sqsel")
        nc.vector.tensor_copy(out=sqsel, in_=ps_t4)

        def bn(name, n=128):
            return bnsb[0:n, BN_COL[name]:BN_COL[name] + 1]

        # ================= PHASE 1: conv1d, batch-emitted by half =========
        # z and sq rows are written straight into the group-major augmented
        # tiles (zm rows 32sg+[0..5] = z, +6 = ones, +7 = sq; zs = -2z / sq
        # at +6 / ones at +7).
        for half in range(2):
            pairs = (2 * half, 2 * half + 1)
            ps1s = {}
            for p in pairs:
                ps1 = pbig.tile([64, T], f32, tag="pbig")
                nc.tensor.matmul(ps1, w1imT, imall[:, p])
                ps1s[p] = ps1
            h1s = {}
            for p in pairs:
                # h1pad2: rows 0-63 = gelu(bn(ps1)) at cols 2..514 (pad 2
                # each side); rows 64-127 = same shifted left 1 col.
                h1pad2 = c1p.tile([128, T + 4], f16, tag="h1pad2")
                if p < 3:
                    nc.vector.memset(h1pad2[:, 0:2], 0.0)
                    nc.vector.memset(h1pad2[:, T + 1:T + 4], 0.0)
                nc.scalar.activation(out=h1pad2[0:64, 2:2 + T], in_=ps1s[p],
                                     func=AF.Gelu, bias=bn("bn1b", 64),
                                     scale=bn("bn1s", 64))
                nc.vector.tensor_copy(out=h1pad2[64:128, 1:1 + T],
                                      in_=h1pad2[0:64, 2:2 + T])
                h1s[p] = h1pad2
            ps2s = {}
            for p in pairs:
                h1pad2 = h1s[p]
                ps2 = pbig.tile([128, T], f32, tag="pbig")
                nc.tensor.matmul(ps2, csb["w2p"][:, 0, :], h1pad2[:, 0:T],
                                 start=True, stop=False)
                nc.tensor.matmul(ps2, csb["w2p"][:, 1, :], h1pad2[:, 2:2 + T],
                                 start=False, stop=False)
                nc.tensor.matmul(ps2, csb["w2p"][0:64, 2, :],
                                 h1pad2[0:64, 4:4 + T],
                                 start=False, stop=True)
                ps2s[p] = ps2
            h2s = {}
            for p in pairs:
                h2pad = c1p.tile([128, T + 2], f16, tag="h2pad")
                if p < 3:
                    nc.vector.memset(h2pad[:, 0:1], 0.0)
                    nc.vector.memset(h2pad[:, T + 1:T + 2], 0.0)
                nc.scalar.activation(out=h2pad[:, 1:1 + T], in_=ps2s[p],
                                     func=AF.Gelu, bias=bn("bn2b"),
                                     scale=bn("bn2s"))
                h2s[p] = h2pad
            ps3s = {}
            for p in pairs:
                ps3 = pbig.tile([38, T], f32, tag="pbig")
                for k in range(3):
                    nc.tensor.matmul(ps3, w3T38[:, k, :], h2s[p][:, k:k + T],
                                     start=(k == 0), stop=(k == 2))
                ps3s[p] = ps3
            for p in pairs:
                g, sgb, h = p // 2, 2 * (p % 2), p % 2
                ps3 = ps3s[p]
                # z / -2z / z^2; psum rows {0:6, 32:38} -> zaug rows 32sg+d
                for s2 in range(2):
                    r0 = 32 * (sgb + s2)
                    nc.vector.tensor_copy(out=zaug_m[g][r0:r0 + 6, :],
                                          in_=ps3[32 * s2:32 * s2 + 6, :])
                    nc.vector.tensor_scalar_mul(out=zaug_s[g][r0:r0 + 6, :],
                                                in0=ps3[32 * s2:32 * s2 + 6, :],
                                                scalar1=-2.0)
                    nc.vector.tensor_mul(out=zsq[64 * h + 32 * s2:
                                                 64 * h + 32 * s2 + 6, :],
                                         in0=zaug_m[g][r0:r0 + 6, :],
                                         in1=zaug_m[g][r0:r0 + 6, :])
                ps_sq = psml.tile([2, T], f32, tag="ps")
                nc.tensor.matmul(ps_sq, sqsel[64 * h:64 * h + 38, :],
                                 zsq[64 * h:64 * h + 38, :],
                                 tile_position=(64 * h, 0))
                nc.vector.tensor_copy(out=sqr_sb[32 * p:32 * p + 2, :],
                                      in_=ps_sq)
                for s2 in range(2):
                    sg = sgb + s2
                    eng = QS[s2 % 2]
                    eng.dma_start(
                        out=zaug_m[g][32 * sg + 7:32 * sg + 8, :],
                        in_=sqr_sb[32 * p + s2:32 * p + s2 + 1, :])
                    eng.dma_start(
                        out=zaug_s[g][32 * sg + 6:32 * sg + 7, :],
                        in_=sqr_sb[32 * p + s2:32 * p + s2 + 1, :])

        # Stage 2: mid/bulky weights, emitted AFTER conv1d so the conv gelu
        # dispatches aren't queued behind hundreds of DMA descriptors on the
        # Act queue (none of these are needed before ~70us).  NEVER on the
        # gpsimd queue: Pool DMAs are software-DGE and eat that sequencer.
        for i, n in enumerate(["m8sel", "fc2bias", "fc1brow", "p025",
                               "c1imT", "fc2wb"]):
            t = ctile(n)
            QS[i % 2].dma_start(out=t, in_=dram[n])
        for n in ["w2p2", "w2s2", "w3p", "w3s", "cw4T", "fc1wT"]:
            t = ctile(n)
            h = _C16_SHAPES[n][0] // 2
            nc.sync.dma_start(out=t[0:h], in_=dram[n][0:h])
            nc.scalar.dma_start(out=t[h:], in_=dram[n][h:])

        # ===== dist matmul + clamp + sqrt + diag-fix, both groups =====
        # (all sqrts emitted before any exp so the Scalar queue never
        # head-of-line-blocks group 1's sqrts behind group 0's exps)
        ecols = {}   # (pair, r) -> (128, 256) f16
        scrs = {}
        for g in range(2):
            for sg in range(4):
                s = 4 * g + sg
                for r in range(4):
                    psd = pbig.tile([128, T], f32, tag="pbig")
                    nc.tensor.matmul(psd,
                                     zaug_s[g][32 * sg:32 * sg + 8,
                                               128 * r:128 * r + 128],
                                     zaug_m[g][32 * sg:32 * sg + 8, :],
                                     tile_position=(32 * sg, 0))
                    dmax = dstp.tile([128, T], f16, tag="dmax", bufs=6)
                    nc.vector.tensor_scalar(out=dmax, in0=psd, scalar1=0.0,
                                            scalar2=1e-6, op0=ALU.max,
                                            op1=ALU.add)
                    scr = dstp.tile([128, T], f16, tag=f"scr_{s}_{r}",
                                    bufs=1, name=f"scr_{s}_{r}")
                    nc.scalar.activation(out=scr, in_=dmax, func=AF.Sqrt,
                                         bias=0.0, scale=1.0,
                                         accum_out=rs[:, s, r:r + 1])
                    # exact diag dist = 1e-3; the diagonal lives at
                    # k in [16r, 16r+16) where col {8k+3+e} == row p + 128r.
                    nc.gpsimd.affine_select(
                        out=scr.rearrange("p (k e) -> p k e", e=8)
                            [:, 16 * r:16 * r + 16, 3:5],
                        in_=scr.rearrange("p (k e) -> p k e", e=8)
                            [:, 16 * r:16 * r + 16, 3:5],
                        compare_op=ALU.not_equal, fill=1e-3,
                        base=-3, pattern=[[-8, 16], [-1, 2]],
                        channel_multiplier=1)
                    scrs[(s, r)] = scr

        # ===== per group: sigma -> exp -> rp pooling -> norm -> imY =====
        imYs = []
        for g in range(2):
            nc.vector.tensor_reduce(out=rrt[:, 4 * g:4 * g + 4],
                                    in_=rs[:, 4 * g:4 * g + 4, :],
                                    axis=mybir.AxisListType.X, op=ALU.add)
            ps_s4 = psml.tile([1, 4], f32, tag="ps")
            nc.tensor.matmul(ps_s4, ones128x1, rrt[:, 4 * g:4 * g + 4])
            sgr = dstp.tile([1, 4], f32, tag="sgr")
            nc.vector.tensor_scalar(out=sgr, in0=ps_s4,
                                    scalar1=-1.0 / (T * T), scalar2=-1e-4,
                                    op0=ALU.mult, op1=ALU.add)
            nc.vector.reciprocal(out=sgr, in_=sgr)
            ps_n4 = psml.tile([128, 4], f32, tag="ps")
            nc.tensor.matmul(ps_n4, ones1x128, sgr)
            nc.vector.tensor_copy(out=nrs[:, 4 * g:4 * g + 4], in_=ps_n4)

            # exp on the strided subgrid columns {8k+3, 8k+4}
            for sg in range(4):
                s = 4 * g + sg
                p_, s2 = divmod(s, 2)
                for r in range(4):
                    if (p_, r) not in ecols:
                        ecols[(p_, r)] = ecolp.tile(
                            [128, 256], f16, tag=f"ecols_{p_}_{r}",
                            name=f"ecols_{p_}_{r}")
                    nc.scalar.activation(
                        out=ecols[(p_, r)][:, 128 * s2:128 * s2 + 128],
                        in_=scrs[(s, r)]
                            .rearrange("p (k e) -> p k e", e=8)[:, :, 3:5],
                        func=AF.Exp, bias=0.0, scale=nrs[:, s:s + 1])

            xpgrp = xpgrps[g]
            mm8 = pairp.tile([64, 8], f32, tag=f"mm8_{g}", name=f"mm8_{g}")
            for q in range(2):
                p = 2 * g + q
                ps_rp = prp.tile([64, 256], f32, tag="prp")
                for r in range(4):
                    nc.tensor.matmul(ps_rp, csb["p025"][:, r, :], ecols[(p, r)],
                                     start=(r == 0), stop=(r == 3))
                rp_sb = pairp.tile([64, 256], f32, tag="rp_sb")
                nc.vector.tensor_copy(out=rp_sb, in_=ps_rp)
                rp64 = pairp.tile([64, 2, 64], f16, tag=f"rp64_{q}",
                                  name=f"rp64_{g}_{q}")
                v = rp_sb.rearrange("p (s k e) -> p s k e", s=2, e=2)
                nc.vector.tensor_tensor(out=rp64, in0=v[:, :, :, 0],
                                        in1=v[:, :, :, 1], op=ALU.add)
                rp64n = pairp.tile([64, 2, 64], f32, tag="rp64n")
                nc.vector.tensor_scalar_mul(out=rp64n, in0=rp64, scalar1=-1.0)
                nc.vector.tensor_reduce(out=mm8[:, 2 * q:2 * q + 2], in_=rp64,
                                        axis=mybir.AxisListType.X, op=ALU.max)
                nc.vector.tensor_reduce(out=mm8[:, 4 + 2 * q:6 + 2 * q],
                                        in_=rp64n,
                                        axis=mybir.AxisListType.X, op=ALU.max)
                for s2 in range(2):
                    eng = nc.sync if s2 == 0 else nc.scalar
                    eng.dma_start(
                        out=xpgrp[2 * q + s2:2 * q + s2 + 1, 0:66 * 66]
                            .rearrange("o (h w) -> o h w", w=66)[:, 1:65, 1:65],
                        in_=rp64[:, s2, :])

            ps_mm = psml.tile([8, 64], f32, tag="ps")
            nc.tensor.matmul(ps_mm, mm8, ident[0:64, 0:64], is_transpose=True)
            mnmx = pairp.tile([8, 1], f32, tag="mnmx")
            nc.vector.tensor_reduce(out=mnmx, in_=ps_mm,
                                    axis=mybir.AxisListType.X, op=ALU.max)
            ps_den = psml.tile([4, 1], f32, tag="ps")
            nc.tensor.matmul(ps_den, csb["m8sel"][:, 0:4], mnmx)
            ps_ngm = psml.tile([4, 1], f32, tag="ps")
            nc.tensor.matmul(ps_ngm, csb["m8sel"][:, 4:8], mnmx)
            sden = pairp.tile([4, 1], f32, tag="sden")
            rcp = pairp.tile([4, 1], f32, tag="rcp")
            ngm = pairp.tile([4, 1], f32, tag="ngm")
            nc.vector.tensor_scalar(out=sden, in0=ps_den, scalar1=1e-4,
                                    scalar2=None, op0=ALU.add, op1=ALU.bypass)
            nc.vector.reciprocal(out=rcp, in_=sden)
            nc.vector.tensor_copy(out=ngm, in_=ps_ngm)
            intv = xpgrp[:, 0:66 * 66].rearrange(
                "o (h w) -> o h w", w=66)[:, 1:65, 1:65]
            nc.vector.tensor_scalar(out=intv, in0=intv, scalar1=ngm,
                                    scalar2=rcp, op0=ALU.add, op1=ALU.mult)

            # L1 im2col bands (9 dy/dx-shifted copies).  Bands 0 and 32 are
            # 32-aligned so the DVE can build them directly (~free); the
            # other 7 spread over all three DMA queues (SBUF->SBUF DMA is
            # ~13GB/s serial per queue, and the Pool queue is idle here).
            imY = l1p.tile([36, 64 * 66], f16, tag=f"imY{g}", name=f"imY{g}")
            imYs.append(imY)
            engs3 = [nc.sync, nc.scalar, nc.gpsimd]
            i = 0
            for dx in range(3):
                for dy in range(3):
                    b = 12 * dx + 4 * dy
                    off = dy * 66 + dx
                    if b in (0, 32):
                        nc.vector.tensor_copy(
                            out=imY[b:b + 4, :],
                            in_=xpgrp[:, off:off + 64 * 66])
                    else:
                        engs3[i % 3].dma_start(
                            out=imY[b:b + 4, :],
                            in_=xpgrp[:, off:off + 64 * 66])
                        i += 1

        # ===== CNN L1 (K=36), both groups =====
        gl1s = []
        for g in range(2):
            imYv = imYs[g].rearrange("p (a b) -> p a b", b=66)
            gl1 = l1p.tile([128, 4096], f16, tag=f"gl1_{g}", name=f"gl1_{g}")
            gl1s.append(gl1)
            for cchunk in range(8):
                psL1 = pbig.tile([128, 512], f32, tag="pbig")
                nc.tensor.matmul(psL1, csb["c1imT"],
                                 imYv[:, 8 * cchunk:8 * cchunk + 8, 0:64])
                nc.scalar.activation(out=gl1[:, 512 * cchunk:512 * cchunk + 512],
                                     in_=psL1, func=AF.Gelu,
                                     bias=bn("cbn1b"), scale=bn("cbn1s"))

        # ===== pool1 into L2 band tiles, both groups =====
        for g in range(2):
            gl1 = gl1s[g]
            pm1 = l1p.tile([128, 64, 32], f16, tag=f"pm1_{g}", name=f"pm1_{g}")
            v1 = gl1.rearrange("p (h w e) -> p h w e", w=32, e=2)
            nc.vector.tensor_tensor(out=pm1, in0=v1[:, :, :, 0],
                                    in1=v1[:, :, :, 1], op=ALU.max)
            v2 = pm1.rearrange("p (h e) w -> p h e w", e=2)
            for q in range(2):
                bt = xl2b[(g, q)]
                nc.vector.tensor_tensor(
                    out=bt[0:64].rearrange("p (a b) -> p a b", b=34)
                        [:, 1:33, 1:33],
                    in0=v2[64 * q:64 * q + 64, :, 0, :],
                    in1=v2[64 * q:64 * q + 64, :, 1, :], op=ALU.max)
                # shifted duplicate band (tap pairs): rows 64-127 = <<1 col
                nc.vector.tensor_copy(out=bt[64:128, 0:34 * 34 - 1],
                                      in_=bt[0:64, 1:34 * 34])

        # ===== CNN L2..L4, interleaved so the PE never waits on pool chains
        def emit_l2(g, q):
            gl2 = l1p.tile([128, 1024], f16, tag=f"gl2_{q}")
            btv = xl2b[(g, q)].rearrange("p (a b) -> p a b", b=34)
            for cchunk in range(2):
                psL2 = pbig.tile([128, 512], f32, tag="pbig")
                h0 = 16 * cchunk
                for dy in range(3):
                    nc.tensor.matmul(
                        psL2, csb["w2p2"][:, dy, :],
                        btv[:, h0 + dy:h0 + dy + 16, 0:32],
                        start=(dy == 0), stop=False)
                for dy in range(3):
                    nc.tensor.matmul(
                        psL2, csb["w2s2"][:, dy, :],
                        btv[0:64, h0 + dy:h0 + dy + 16, 2:34],
                        start=False, stop=(dy == 2))
                nc.scalar.activation(
                    out=gl2[:, 512 * cchunk:512 * cchunk + 512], in_=psL2,
                    func=AF.Gelu, bias=bn("cbn2b"), scale=bn("cbn2s"))

            # maxpool 32x32 -> 16x16 into the per-sample L3 band tiles
            pm2 = l1p.tile([128, 32, 16], f16, tag=f"pm2_{q}")
            w1v = gl2.rearrange("p (h w e) -> p h w e", w=16, e=2)
            nc.vector.tensor_tensor(out=pm2, in0=w1v[:, :, :, 0],
                                    in1=w1v[:, :, :, 1], op=ALU.max)
            w2v = pm2.rearrange("p (h e) w -> p h e w", e=2)
            for s2 in range(2):
                bt3 = xl3b[(g, q, s2)]
                nc.vector.tensor_tensor(
                    out=bt3[0:64].rearrange("p (a b) -> p a b", b=18)
                        [:, 1:17, 1:17],
                    in0=w2v[64 * s2:64 * s2 + 64, :, 0, :],
                    in1=w2v[64 * s2:64 * s2 + 64, :, 1, :], op=ALU.max)
                nc.vector.tensor_copy(out=bt3[64:128, 0:18 * 18 - 1],
                                      in_=bt3[0:64, 1:18 * 18])

        def emit_l3(g, q, s2):
            sg = 2 * q + s2
            bt3v = xl3b[(g, q, s2)].rearrange("p (a b) -> p a b", b=18)
            psL3 = pbig.tile([128, 256], f32, tag="pbig")
            for dy in range(3):
                nc.tensor.matmul(psL3, csb["w3p"][:, dy, :],
                                 bt3v[:, dy:dy + 16, 0:16],
                                 start=(dy == 0), stop=False)
            for dy in range(3):
                nc.tensor.matmul(psL3, csb["w3s"][:, dy, :],
                                 bt3v[0:64, dy:dy + 16, 2:18],
                                 start=False, stop=(dy == 2))
            gl3 = l1p.tile([128, 256], f16, tag=f"gl3_{s2}")
            nc.scalar.activation(out=gl3, in_=psL3, func=AF.Gelu,
                                 bias=bn("cbn3b"), scale=bn("cbn3s"))
            # maxpool 16x16 -> 8x8 into l4in (10x10 padded)
            pm3 = l1p.tile([128, 16, 8], f16, tag=f"pm3_{s2}")
            u1 = gl3.rearrange("p (h w e) -> p h w e", w=8, e=2)
            nc.vector.tensor_tensor(out=pm3, in0=u1[:, :, :, 0],
                                    in1=u1[:, :, :, 1], op=ALU.max)
            u2 = pm3.rearrange("p (h e) w -> p h e w", e=2)
            nc.vector.tensor_tensor(
                out=l4ins[g].rearrange("p (s a b) -> p s a b", a=10, b=10)
                    [:, sg, 1:9, 1:9],
                in0=u2[:, :, 0, :], in1=u2[:, :, 1, :], op=ALU.max)

        def emit_l4(g):
            psL4 = pbig.tile([128, 256], f32, tag="pbig")
            xl4 = l4ins[g].rearrange("p (s a b) -> p s a b", a=10, b=10)
            for t in range(9):
                dy, dx = t // 3, t % 3
                nc.tensor.matmul(psL4, csb["cw4T"][:, t, :],
                                 xl4[:, :, dy:dy + 8, dx:dx + 8],
                                 start=(t == 0), stop=(t == 8))
            gl4 = l1p.tile([128, 256], f16, tag="gl4")
            nc.scalar.activation(out=gl4, in_=psL4, func=AF.Gelu,
                                 bias=bn("cbn4b"), scale=bn("cbn4s"))
            # avgpool 8x8 -> 4x4 (sum; 0.25 folded into fc1 weights)
            av1 = l1p.tile([128, 128], f16, tag="av1")
            a1 = gl4.rearrange("p (s h w e) -> p s h w e", s=4, w=4, e=2)
            nc.vector.tensor_tensor(
                out=av1.rearrange("p (s h w) -> p s h w", s=4, w=4),
                in0=a1[:, :, :, :, 0], in1=a1[:, :, :, :, 1], op=ALU.add)
            a2 = av1.rearrange("p (s h e w) -> p s h e w", s=4, e=2, w=4)
            nc.vector.tensor_tensor(out=fcin[:, 64 * g:64 * g + 64]
                                        .rearrange("p (s h w) -> p s h w", s=4, w=4),
                                    in0=a2[:, :, :, 0, :], in1=a2[:, :, :, 1, :],
                                    op=ALU.add)

        emit_l2(0, 0)
        emit_l2(0, 1)
        emit_l3(0, 0, 0)
        emit_l3(0, 0, 1)
        emit_l3(0, 1, 0)
        emit_l3(0, 1, 1)
        emit_l2(1, 0)
        emit_l2(1, 1)
        emit_l4(0)
        emit_l3(1, 0, 0)
        emit_l3(1, 0, 1)
        emit_l3(1, 1, 0)
        emit_l3(1, 1, 1)
        emit_l4(1)

        # ================= FC head =================
        ps_fc = prp.tile([8, 256], f32, tag="prp")
        fv = fcin.rearrange("p (s j) -> p s j", j=16)
        for j in range(16):
            nc.tensor.matmul(ps_fc, fv[:, :, j], csb["fc1wT"][:, j, :],
                             start=(j == 0), stop=False)
        nc.tensor.matmul(ps_fc, onesK1M8, csb["fc1brow"], start=False, stop=True)
        nc.scalar.activation(out=fch, in_=ps_fc, func=AF.Gelu)
        junk = sing.tile([8, 256], f32)
        res8 = sing.tile([8, 1], f32)
        nc.vector.scalar_tensor_tensor(out=junk, in0=fch, scalar=1.0,
                                       in1=csb["fc2wb"], op0=ALU.mult,
                                       op1=ALU.mult, accum_out=res8)
        res8b = sing.tile([8, 1], f32)
        nc.vector.tensor_tensor(out=res8b, in0=res8, in1=csb["fc2bias"],
                                op=ALU.add)
        nc.sync.dma_start(out=out, in_=res8b)


# ------------------------------------------------------------------ driver
_prog_cache = {}


def _get_program(debug=False):
    key = ("dbg" if debug else "main")
    if key not in _prog_cache:
        _prog_cache[key] = build_program(debug=debug)
    return _prog_cache[key]


def _im2col_x(xs):
    """(8, 8, 512) f32 -> (112, 4, 512) f16 conv1d-1 im2col.

    Partition row 16k + 8s2 + c, pair p, col t = xpad[2p + s2, c, t + k]
    (pad 3 left/right).
    """
    xp = np.zeros((SPC, 8, T + 6), np.float16)
    xp[:, :, 3:3 + T] = xs.astype(np.float16)
    im = np.empty((7, 2, 8, 4, T), np.float16)
    for k in range(7):
        v = xp[:, :, k:k + T].reshape(4, 2, 8, T)
        im[k] = v.transpose(1, 2, 0, 3)
    return np.ascontiguousarray(im.reshape(112, 4, T))


def _run(inputs, debug=False):
    x = np.ascontiguousarray(np.asarray(inputs["x"]), np.float32)
    assert x.shape == (64, 8, 512), x.shape
    consts = _pack_consts({k: np.asarray(v) for k, v in inputs.items()})
    nc = _get_program(debug=debug)
    in_maps = []
    for c in range(N_CORES):
        m = dict(consts)
        m["xim"] = _im2col_x(x[SPC * c:SPC * c + SPC])
        in_maps.append(m)
    return run_bass_kernel_spmd(nc, in_maps, list(range(N_CORES)))


def kernel(**inputs):
    res = _run(inputs, debug=False)
    return np.concatenate([res.results[c]["out"][:, 0] for c in range(N_CORES)])


# revision 58
# speedup vs baseline: 1.1876x; 1.1876x over previous
"""Trainium2 Bass kernel for nn_EndToEndRPModel.

Pipeline per sample: conv1d stack (8ch,T=512 -> 6ch) -> pairwise-distance
soft recurrence plot (512x512) -> bilinear resize to 64x64 (exact 2x2 mean
of a strided 128x128 subgrid since scale=8) -> min-max norm -> small CNN ->
FC head -> scalar.

Sharding: pure data parallel, 8 samples per core on 8 cores.

Key implementation notes:
 - all heavy matmuls run in fp16 (1 cyc/col); d2 = sq_i + sq_j - 2*gram via
   ONE augmented fp16 matmul per 128-row tile, 4 samples packed into
   disjoint PE row quadrants via tile_position.
 - d2 diagonal forced to dist=1e-3 with gpsimd.affine_select restricted to
   the 16-col diagonal window of each row tile.
 - bilinear(512->64) == 0.25 * 2x2-sum over rows/cols {8j+3, 8j+4}; row
   selection+0.25 folded into a pooling matmul, col selection into the exp.
 - conv1d-2 / CNN L2 / CNN L3 run as tap-pair matmuls (K=128: two
   column-shifted copies of the input stacked in the partition dim),
   plus one single-tap matmul for the odd tap column.
 - CNN L1 runs as K=36 matmuls with all 9 taps baked into 9 dy/dx-shifted
   partition bands of the im2col tile (built with cheap vector copies).
 - sigma chain batched per group (one reduce + 2 tiny matmuls for 4
   samples).
 - narrow constants ship transposed (few wide DMA descriptors) and are
   transposed back on the PE at startup; identities/memsets are emitted
   before any Pool-queue DMA so nothing blocks them.
 - all BN affines folded into the Gelu activation's per-partition
   scale/bias; avgpool's 0.25 folded into the FC1 weights.
"""
import sys

sys.path.insert(0, "/opt/trn_rl_repo")

import numpy as np

import concourse.bacc as bacc
import concourse.tile as tile
from concourse import mybir
from concourse.bass_utils import run_bass_kernel_spmd
from concourse.masks import make_identity

f32 = mybir.dt.float32
f32r = mybir.dt.float32r
f16 = mybir.dt.float16
AF = mybir.ActivationFunctionType
ALU = mybir.AluOpType

N_CORES = 8
SPC = 8          # samples per core
T = 512
BN_KAPPA = 1.0 / np.sqrt(1.0 + 1e-5)


# ---------------------------------------------------------------- host-side
def _pack_consts(inp):
    """Pack all weights into the exact SBUF layouts the kernel uses."""
    c16 = {}
    c32 = {}
    w1 = inp["w1"]; w2 = inp["w2"]; w3 = inp["w3"]

    # conv1d-1 im2col weights: rows 16k + 8s2 + ch, cols 32s2 + o
    # shipped transposed [64, 112] and PE-transposed on chip.
    w1imT = np.zeros((112, 64), np.float32)
    for k in range(7):
        for s2 in range(2):
            w1imT[16 * k + 8 * s2:16 * k + 8 * s2 + 8, 32 * s2:32 * s2 + 32] = \
                w1[:, :, k].T
    c16["w1imTT"] = np.ascontiguousarray(w1imT.T)

    # conv1d-2 tap-pair weights: rounds (0,1), (2,3) are [128, 128]
    # (rows 64b + 32s2 + ch for band b in {tap k, tap k+1}); round 4 is
    # [64, 128] single-tap.  Shipped as one [128, 3, 128] tensor.
    w2p = np.zeros((128, 3, 128), np.float32)
    for rnd, k0 in enumerate((0, 2)):
        for b in range(2):
            for s2 in range(2):
                w2p[64 * b + 32 * s2:64 * b + 32 * s2 + 32, rnd,
                    64 * s2:64 * s2 + 64] = w2[:, :, k0 + b].T
    for s2 in range(2):
        w2p[32 * s2:32 * s2 + 32, 2, 64 * s2:64 * s2 + 64] = w2[:, :, 4].T
    c16["w2p"] = w2p

    # conv1d-3 taps: (128, 3, 12): rows 64s2+ch, cols 6s2+d
    # shipped transposed [36, 128] (rows 12k + 6s2 + d) and PE-transposed,
    # then scattered into even/odd-pair lhsT tiles on chip (z output lands
    # at 32-aligned psum rows 32sg+d so DVE can copy it).
    w3T = np.zeros((128, 3, 12), np.float32)
    for k in range(3):
        for s2 in range(2):
            w3T[64 * s2:64 * s2 + 64, k, 6 * s2:6 * s2 + 6] = w3[:, :, k].T
    c16["w3TT"] = np.ascontiguousarray(w3T.reshape(128, 36).T)

    # sq selector: rows 32sg + d -> col s2 (even/odd pair via 64-row halves)
    sqsel = np.zeros((128, 2), np.float32)
    for h in range(2):
        for s2 in range(2):
            sqsel[64 * h + 32 * s2:64 * h + 32 * s2 + 6, s2] = 1.0
    c32["sqselT"] = np.ascontiguousarray(sqsel.T)

    # pooling matrix for rp row-pairs: p025[p, r, j] = 0.25 if 128r+p in {8j+3, 8j+4}
    p025 = np.zeros((128, 4, 64), np.float32)
    for r in range(4):
        for p in range(128):
            i = 128 * r + p
            if i % 8 in (3, 4):
                j = (i - 3) // 8 if i % 8 == 3 else (i - 4) // 8
                if 0 <= j < 64:
                    p025[p, r, j] = 0.25
    c16["p025"] = p025

    # min-max combiner: mnmx8 rows = [mx0..mx3, -mn0..-mn3]
    m8 = np.zeros((8, 8), np.float32)
    for s in range(4):
        m8[s, s] = m8[4 + s, s] = 1.0    # den_s = mx_s + (-mn_s)
        m8[4 + s, 4 + s] = 1.0           # negmn_s
    c32["m8sel"] = m8

    # 2D conv weights
    c1 = inp["c1"]; c2 = inp["c2"]; c3 = inp["c3"]; c4 = inp["c4"]
    # L1: K=36 im2col, rows 12dx + 4dy + s, cols 32s + o
    c1imT = np.zeros((36, 128), np.float32)
    for s in range(4):
        for dy in range(3):
            for dx in range(3):
                c1imT[12 * dx + 4 * dy + s, 32 * s:32 * s + 32] = c1[:, 0, dy, dx]
    c16["c1imT"] = c1imT

    # L2 tap-pair weights: bands [s0, s1, s0<<1col, s1<<1col] x 32ch
    w2p2 = np.zeros((128, 3, 128), np.float32)
    w2s2 = np.zeros((64, 3, 128), np.float32)
    for dy in range(3):
        for s2 in range(2):
            w2p2[32 * s2:32 * s2 + 32, dy, 64 * s2:64 * s2 + 64] = \
                c2[:, :, dy, 0].T
            w2p2[64 + 32 * s2:64 + 32 * s2 + 32, dy, 64 * s2:64 * s2 + 64] = \
                c2[:, :, dy, 1].T
            w2s2[32 * s2:32 * s2 + 32, dy, 64 * s2:64 * s2 + 64] = \
                c2[:, :, dy, 2].T
    c16["w2p2"] = w2p2
    c16["w2s2"] = w2s2

    # L3 tap-pair weights: bands [64ch, 64ch<<1col]
    w3p = np.zeros((128, 3, 128), np.float32)
    w3s = np.zeros((64, 3, 128), np.float32)
    for dy in range(3):
        w3p[0:64, dy, :] = c3[:, :, dy, 0].T
        w3p[64:128, dy, :] = c3[:, :, dy, 1].T
        w3s[:, dy, :] = c3[:, :, dy, 2].T
    c16["w3p"] = w3p
    c16["w3s"] = w3s

    cw4T = np.zeros((128, 9, 128), np.float32)
    for t in range(9):
        dy, dx = t // 3, t % 3
        cw4T[:, t, :] = c4[:, :, dy, dx].T
    c16["cw4T"] = cw4T

    # FC1 weights: (128, 16, 256), 0.25 avgpool folded in
    fc1_w = np.asarray(inp["fc1_w"], np.float32)        # (256, 2048)
    c16["fc1wT"] = 0.25 * np.ascontiguousarray(
        fc1_w.reshape(256, 128, 16).transpose(1, 2, 0))
    c16["fc1brow"] = inp["fc1_b"].reshape(1, 256).astype(np.float32)
    c32["fc2wb"] = np.broadcast_to(
        inp["fc2_w"].reshape(1, 256), (8, 256)).astype(np.float32).copy()
    c32["fc2bias"] = np.full(
        (8, 1), float(np.asarray(inp["fc2_b"]).reshape(-1)[0]), np.float32)

    # BN scale/bias vectors, one [16, 128] f32 pack shipped transposed.
    def rep(v, reps):
        return np.tile(np.asarray(v, np.float32), reps)
    bnT = np.zeros((16, 128), np.float32)
    bnT[0] = rep(inp["g1"] * BN_KAPPA, 4); bnT[1] = rep(inp["b1"], 4)
    bnT[2] = rep(inp["g2"] * BN_KAPPA, 2); bnT[3] = rep(inp["b2"], 2)
    bnT[4] = rep(inp["cg1"] * BN_KAPPA, 4); bnT[5] = rep(inp["cb1"], 4)
    bnT[6] = rep(inp["cg2"] * BN_KAPPA, 2); bnT[7] = rep(inp["cb2"], 2)
    bnT[8] = inp["cg3"] * BN_KAPPA; bnT[9] = inp["cb3"]
    bnT[10] = inp["cg4"] * BN_KAPPA; bnT[11] = inp["cb4"]
    c32["bnT"] = bnT

    out = {k: np.ascontiguousarray(v, np.float16) for k, v in c16.items()}
    out.update({k: np.ascontiguousarray(v, np.float32) for k, v in c32.items()})
    return out


# ------------------------------------------------------------- bass program
_C16_SHAPES = {
    "w1imTT": (64, 112), "w2p": (128, 3, 128), "w3TT": (36, 128),
    "p025": (128, 4, 64), "c1imT": (36, 128), "w2p2": (128, 3, 128),
    "w2s2": (64, 3, 128), "w3p": (128, 3, 128), "w3s": (64, 3, 128),
    "cw4T": (128, 9, 128), "fc1wT": (128, 16, 256), "fc1brow": (1, 256),
}
_C32_SHAPES = {
    "sqselT": (2, 128), "m8sel": (8, 8), "fc2wb": (8, 256), "fc2bias": (8, 1),
    "bnT": (16, 128),
}

BN_COL = {"bn1s": 0, "bn1b": 1, "bn2s": 2, "bn2b": 3, "cbn1s": 4, "cbn1b": 5,
          "cbn2s": 6, "cbn2b": 7, "cbn3s": 8, "cbn3b": 9, "cbn4s": 10,
          "cbn4b": 11}


def build_program(debug=False):
    nc = bacc.Bacc("TRN2", target_bir_lowering=False, debug=False,
                   num_devices=N_CORES)
    xim = nc.dram_tensor("xim", [112, 4, T], f16, kind="ExternalInput").ap()
    dram = {n: nc.dram_tensor(n, list(s), f16, kind="ExternalInput").ap()
            for n, s in _C16_SHAPES.items()}
    dram.update({n: nc.dram_tensor(n, list(s), f32, kind="ExternalInput").ap()
                 for n, s in _C32_SHAPES.items()})
    out = nc.dram_tensor("out", [SPC, 1], f32, kind="ExternalOutput").ap()

    with tile.TileContext(nc) as tc:
        _emit(tc, nc, xim, dram, out)
    nc.compile()
    return nc


def _emit(tc, nc, xim, dram, out):
    from contextlib import ExitStack
    ctx = ExitStack()
    with ctx:
        cpool = ctx.enter_context(tc.tile_pool(name="consts", bufs=1))
        sing = ctx.enter_context(tc.tile_pool(name="sing", bufs=1))
        c1p = ctx.enter_context(tc.tile_pool(name="conv1", bufs=3))
        dstp = ctx.enter_context(tc.tile_pool(name="dist", bufs=3))
        pairp = ctx.enter_context(tc.tile_pool(name="pairs", bufs=2))
        ecolp = ctx.enter_context(tc.tile_pool(name="ecols", bufs=1))
        grpp = ctx.enter_context(tc.tile_pool(name="grp", bufs=1))
        l1p = ctx.enter_context(tc.tile_pool(name="lcnn", bufs=1))
        pbig = ctx.enter_context(tc.tile_pool(name="pbig", bufs=6, space="PSUM"))
        prp = ctx.enter_context(tc.tile_pool(name="prp", bufs=1, space="PSUM"))
        psml = ctx.enter_context(tc.tile_pool(name="psml", bufs=1, space="PSUM"))

        # ------------- persistent tiles (allocated before anything runs)
        zaug_m = [grpp.tile([128, T], f16, tag=f"zaug_m{g}", name=f"zaug_m{g}")
                  for g in range(2)]
        zaug_s = [grpp.tile([128, T], f16, tag=f"zaug_s{g}", name=f"zaug_s{g}")
                  for g in range(2)]
        # 2 extra cols so the (dy=2, dx=2) L1 im2col band read stays in range
        xpgrps = [grpp.tile([4, 66 * 66 + 2], f16, tag=f"xpg{g}",
                            name=f"xpg{g}") for g in range(2)]
        xl2b = {(g, q): l1p.tile([128, 34 * 34], f16, tag=f"xl2b_{g}_{q}",
                                 name=f"xl2b_{g}_{q}")
                for g in range(2) for q in range(2)}
        xl3b = {(g, q, s2): l1p.tile([128, 18 * 18], f16,
                                     tag=f"xl3b_{g}_{q}_{s2}",
                                     name=f"xl3b_{g}_{q}_{s2}")
                for g in range(2) for q in range(2) for s2 in range(2)}
        zsq = sing.tile([128, T], f16)
        sqr_sb = sing.tile([128, T], f16)     # pair p sq rows at 32p, 32p+1
        rs = sing.tile([128, 8, 4], f32)       # sqrt row-sums per (s, r)
        rrt = sing.tile([128, 8], f32)
        nrs = sing.tile([128, 8], f32)         # -1/sigma broadcast per sample
        fcin = sing.tile([128, 128], f16)
        fch = sing.tile([8, 256], f32)

        # ------------- setup on the (otherwise idle) gpsimd queue so the
        # vector queue stays free for the startup const copies.  Order:
        # identities (gate the PE transposes), then tiles needed by conv1d,
        # then the late-phase tiles.  zaug ones-rows are filled as [8, T]
        # 32-aligned strips: value!=0 memsets are ~10x slower than 0.0 and
        # cost scales with rows, and later z/-2z/sq writes overwrite 7 of 8.
        ident = cpool.tile([128, 128], f32)
        make_identity(nc, ident)
        identh = cpool.tile([64, 64], f16)
        make_identity(nc, identh)
        ones128x1 = cpool.tile([128, 1], f32)
        nc.gpsimd.memset(ones128x1, 1.0)
        ones1x128 = cpool.tile([1, 128], f32)
        nc.gpsimd.memset(ones1x128, 1.0)
        # conv1d-3 lhsT (cols 0:6 / 32:38 so both samples' z lands at
        # 32-aligned psum rows for every pair)
        w3T38 = cpool.tile([128, 3, 38], f16, name="w3T38")
        nc.gpsimd.memset(w3T38, 0.0)
        for g in range(2):
            for sg in range(4):
                nc.gpsimd.memset(zaug_m[g][32 * sg:32 * sg + 8, :], 1.0)
                nc.gpsimd.memset(zaug_s[g][32 * sg:32 * sg + 8, :], 1.0)
        nc.gpsimd.memset(zsq, 0.0)
        for g in range(2):
            nc.gpsimd.memset(xpgrps[g], 0.0)
        for t in xl2b.values():
            nc.gpsimd.memset(t, 0.0)
        for t in xl3b.values():
            nc.gpsimd.memset(t, 0.0)
        l4ins = [l1p.tile([128, 400], f16, tag=f"l4in{g}", name=f"l4in{g}")
                 for g in range(2)]
        nc.gpsimd.memset(l4ins[0], 0.0)
        nc.gpsimd.memset(l4ins[1], 0.0)

        # ---------------- constants into SBUF (sync + scalar queues)
        csb = {}

        def ctile(n):
            shape = _C16_SHAPES.get(n) or _C32_SHAPES[n]
            t = cpool.tile(list(shape), f16 if n in _C16_SHAPES else f32,
                           name="c_" + n, tag="c_" + n)
            csb[n] = t
            return t

        # Stage 0: first-matmul critical
        t_w1 = ctile("w1imTT")
        nc.sync.dma_start(out=t_w1[0:32], in_=dram["w1imTT"][0:32])
        nc.scalar.dma_start(out=t_w1[32:64], in_=dram["w1imTT"][32:64])
        imall = c1p.tile([112, 4, T], f16, tag="imall", bufs=1, name="imall")
        nc.sync.dma_start(out=imall[0:56, 0], in_=xim[0:56, 0])
        nc.scalar.dma_start(out=imall[56:112, 0], in_=xim[56:112, 0])

        # Stage 1: phase-1 weights + remaining input pairs
        t_bnT = ctile("bnT")
        nc.scalar.dma_start(out=t_bnT, in_=dram["bnT"])
        t_w2p = ctile("w2p")
        nc.sync.dma_start(out=t_w2p[0:64], in_=dram["w2p"][0:64])
        nc.scalar.dma_start(out=t_w2p[64:128], in_=dram["w2p"][64:128])
        t_w3 = ctile("w3TT")
        nc.sync.dma_start(out=t_w3, in_=dram["w3TT"])
        t_sq = ctile("sqselT")
        nc.scalar.dma_start(out=t_sq, in_=dram["sqselT"])
        for p in range(1, 4):
            nc.sync.dma_start(out=imall[0:56, p], in_=xim[0:56, p])
            nc.scalar.dma_start(out=imall[56:112, p], in_=xim[56:112, p])

        # Stage 2a: small/mid weights.  The Act queue gets only 16
        # descriptors here — gelu dispatch queues behind everything ahead
        # of it on the Act queue, so keep that queue on a DMA diet.
        QS = [nc.sync, nc.scalar]
        for n in ["m8sel", "fc1brow", "p025", "c1imT"]:
            t = ctile(n)
            nc.sync.dma_start(out=t, in_=dram[n])
        for n in ["fc2bias", "fc2wb"]:
            t = ctile(n)
            nc.scalar.dma_start(out=t, in_=dram[n])

        onesK1M8 = cpool.tile([1, 8], f16)
        nc.gpsimd.memset(onesK1M8, 1.0)

        # --------- on-chip transposes of narrow consts (also warms the PE)
        ps_t1 = psml.tile([112, 64], f16, tag="ps")
        nc.tensor.matmul(ps_t1, csb["w1imTT"], identh, is_transpose=True)
        w1imT = cpool.tile([112, 64], f16, name="w1imT")
        nc.vector.tensor_copy(out=w1imT, in_=ps_t1)

        ps_t2 = psml.tile([128, 36], f16, tag="ps")
        nc.tensor.matmul(ps_t2, csb["w3TT"], identh[0:36, 0:36],
                         is_transpose=True)
        w3Tsb = cpool.tile([128, 36], f16, name="w3Tsb")
        nc.vector.tensor_copy(out=w3Tsb, in_=ps_t2)
        w3v = w3Tsb.rearrange("p (k sd) -> p k sd", sd=12)
        for k in range(3):
            nc.vector.tensor_copy(out=w3T38[:, k, 0:6], in_=w3v[:, k, 0:6])
            nc.vector.tensor_copy(out=w3T38[:, k, 32:38], in_=w3v[:, k, 6:12])

        ps_t3 = psml.tile([128, 16], f32, tag="ps")
        nc.tensor.matmul(ps_t3, csb["bnT"], ident[0:16, 0:16],
                         is_transpose=True)
        bnsb = cpool.tile([128, 16], f32, name="bnsb")
        nc.vector.tensor_copy(out=bnsb, in_=ps_t3)

        ps_t4 = psml.tile([128, 2], f32, tag="ps")
        nc.tensor.matmul(ps_t4, csb["sqselT"], ident[0:2, 0:2],
                         is_transpose=True)
        sqsel = cpool.tile([128, 2], f16, name="sqsel")
        nc.vector.tensor_copy(out=sqsel, in_=ps_t4)

        def bn(name, n=128):
            return bnsb[0:n, BN_COL[name]:BN_COL[name] + 1]

        # ================= PHASE 1: conv1d, batch-emitted by half =========
        # z and sq rows are written straight into the group-major augmented
        # tiles (zm rows 32sg+[0..5] = z, +6 = ones, +7 = sq; zs = -2z / sq
        # at +6 / ones at +7).
        for half in range(2):
            pairs = (2 * half, 2 * half + 1)
            ps1s = {}
            for p in pairs:
                ps1 = pbig.tile([64, T], f32, tag="pbig")
                nc.tensor.matmul(ps1, w1imT, imall[:, p])
                ps1s[p] = ps1
            h1s = {}
            for p in pairs:
                # h1pad2: rows 0-63 = gelu(bn(ps1)) at cols 2..514 (pad 2
                # each side); rows 64-127 = same shifted left 1 col.
                h1pad2 = c1p.tile([128, T + 4], f16, tag="h1pad2")
                if p < 3:
                    nc.vector.memset(h1pad2[:, 0:2], 0.0)
                    nc.vector.memset(h1pad2[:, T + 1:T + 4], 0.0)
                nc.scalar.activation(out=h1pad2[0:64, 2:2 + T], in_=ps1s[p],
                                     func=AF.Gelu, bias=bn("bn1b", 64),
                                     scale=bn("bn1s", 64))
                nc.vector.tensor_copy(out=h1pad2[64:128, 1:1 + T],
                                      in_=h1pad2[0:64, 2:2 + T])
                h1s[p] = h1pad2
            ps2s = {}
            for p in pairs:
                h1pad2 = h1s[p]
                ps2 = pbig.tile([128, T], f32, tag="pbig")
                nc.tensor.matmul(ps2, csb["w2p"][:, 0, :], h1pad2[:, 0:T],
                                 start=True, stop=False)
                nc.tensor.matmul(ps2, csb["w2p"][:, 1, :], h1pad2[:, 2:2 + T],
                                 start=False, stop=False)
                nc.tensor.matmul(ps2, csb["w2p"][0:64, 2, :],
                                 h1pad2[0:64, 4:4 + T],
                                 start=False, stop=True)
                ps2s[p] = ps2
            h2s = {}
            for p in pairs:
                h2pad = c1p.tile([128, T + 2], f16, tag="h2pad")
                if p < 3:
                    nc.vector.memset(h2pad[:, 0:1], 0.0)
                    nc.vector.memset(h2pad[:, T + 1:T + 2], 0.0)
                nc.scalar.activation(out=h2pad[:, 1:1 + T], in_=ps2s[p],
                                     func=AF.Gelu, bias=bn("bn2b"),
                                     scale=bn("bn2s"))
                h2s[p] = h2pad
            ps3s = {}
            for p in pairs:
                ps3 = pbig.tile([38, T], f32, tag="pbig")
                for k in range(3):
                    nc.tensor.matmul(ps3, w3T38[:, k, :], h2s[p][:, k:k + T],
                                     start=(k == 0), stop=(k == 2))
                ps3s[p] = ps3
            for p in pairs:
                g, sgb, h = p // 2, 2 * (p % 2), p % 2
                ps3 = ps3s[p]
                # z / -2z / z^2; psum rows {0:6, 32:38} -> zaug rows 32sg+d
                for s2 in range(2):
                    r0 = 32 * (sgb + s2)
                    nc.vector.tensor_copy(out=zaug_m[g][r0:r0 + 6, :],
                                          in_=ps3[32 * s2:32 * s2 + 6, :])
                    nc.vector.tensor_scalar_mul(out=zaug_s[g][r0:r0 + 6, :],
                                                in0=ps3[32 * s2:32 * s2 + 6, :],
                                                scalar1=-2.0)
                    nc.vector.tensor_mul(out=zsq[64 * h + 32 * s2:
                                                 64 * h + 32 * s2 + 6, :],
                                         in0=zaug_m[g][r0:r0 + 6, :],
                                         in1=zaug_m[g][r0:r0 + 6, :])
                ps_sq = psml.tile([2, T], f32, tag="ps")
                nc.tensor.matmul(ps_sq, sqsel[64 * h:64 * h + 38, :],
                                 zsq[64 * h:64 * h + 38, :],
                                 tile_position=(64 * h, 0))
                nc.vector.tensor_copy(out=sqr_sb[32 * p:32 * p + 2, :],
                                      in_=ps_sq)
                for s2 in range(2):
                    sg = sgb + s2
                    eng = QS[s2 % 2]
                    eng.dma_start(
                        out=zaug_m[g][32 * sg + 7:32 * sg + 8, :],
                        in_=sqr_sb[32 * p + s2:32 * p + s2 + 1, :])
                    eng.dma_start(
                        out=zaug_s[g][32 * sg + 6:32 * sg + 7, :],
                        in_=sqr_sb[32 * p + s2:32 * p + s2 + 1, :])

        # Stage 2b: bulky CNN/FC weights, emitted after conv1d and entirely
        # on sync: the Act queue must stay empty between the conv gelus and
        # the dist sqrts (H-regression lesson), and sync's later DMAs here
        # (sq-rows above: 1 descriptor each; scatters: needed ~75us) still
        # dispatch in time behind these ~640 descriptors.
        for n in ["w2p2", "w2s2", "w3p", "w3s", "cw4T", "fc1wT"]:
            t = ctile(n)
            nc.sync.dma_start(out=t, in_=dram[n])

        # ===== dist matmul + clamp + sqrt + diag-fix, both groups =====
        # (all sqrts emitted before any exp so the Scalar queue never
        # head-of-line-blocks group 1's sqrts behind group 0's exps)
        ecols = {}   # (pair, r) -> (128, 256) f16
        scrs = {}
        for g in range(2):
            for sg in range(4):
                s = 4 * g + sg
                for r in range(4):
                    psd = pbig.tile([128, T], f32, tag="pbig")
                    nc.tensor.matmul(psd,
                                     zaug_s[g][32 * sg:32 * sg + 8,
                                               128 * r:128 * r + 128],
                                     zaug_m[g][32 * sg:32 * sg + 8, :],
                                     tile_position=(32 * sg, 0))
                    dmax = dstp.tile([128, T], f16, tag="dmax", bufs=6)
                    nc.vector.tensor_scalar(out=dmax, in0=psd, scalar1=0.0,
                                            scalar2=1e-6, op0=ALU.max,
                                            op1=ALU.add)
                    scr = dstp.tile([128, T], f16, tag=f"scr_{s}_{r}",
                                    bufs=1, name=f"scr_{s}_{r}")
                    nc.scalar.activation(out=scr, in_=dmax, func=AF.Sqrt,
                                         bias=0.0, scale=1.0,
                                         accum_out=rs[:, s, r:r + 1])
                    # exact diag dist = 1e-3; the diagonal lives at
                    # k in [16r, 16r+16) where col {8k+3+e} == row p + 128r.
                    nc.gpsimd.affine_select(
                        out=scr.rearrange("p (k e) -> p k e", e=8)
                            [:, 16 * r:16 * r + 16, 3:5],
                        in_=scr.rearrange("p (k e) -> p k e", e=8)
                            [:, 16 * r:16 * r + 16, 3:5],
                        compare_op=ALU.not_equal, fill=1e-3,
                        base=-3, pattern=[[-8, 16], [-1, 2]],
                        channel_multiplier=1)
                    scrs[(s, r)] = scr

        # ===== per group: sigma -> exp -> rp pooling -> norm -> imY =====
        imYs = []
        for g in range(2):
            nc.vector.tensor_reduce(out=rrt[:, 4 * g:4 * g + 4],
                                    in_=rs[:, 4 * g:4 * g + 4, :],
                                    axis=mybir.AxisListType.X, op=ALU.add)
            ps_s4 = psml.tile([1, 4], f32, tag="ps")
            nc.tensor.matmul(ps_s4, ones128x1, rrt[:, 4 * g:4 * g + 4])
            sgr = dstp.tile([1, 4], f32, tag="sgr")
            nc.vector.tensor_scalar(out=sgr, in0=ps_s4,
                                    scalar1=-1.0 / (T * T), scalar2=-1e-4,
                                    op0=ALU.mult, op1=ALU.add)
            nc.vector.reciprocal(out=sgr, in_=sgr)
            ps_n4 = psml.tile([128, 4], f32, tag="ps")
            nc.tensor.matmul(ps_n4, ones1x128, sgr)
            nc.vector.tensor_copy(out=nrs[:, 4 * g:4 * g + 4], in_=ps_n4)

            # exp on the strided subgrid columns {8k+3, 8k+4}
            for sg in range(4):
                s = 4 * g + sg
                p_, s2 = divmod(s, 2)
                for r in range(4):
                    if (p_, r) not in ecols:
                        ecols[(p_, r)] = ecolp.tile(
                            [128, 256], f16, tag=f"ecols_{p_}_{r}",
                            name=f"ecols_{p_}_{r}")
                    nc.scalar.activation(
                        out=ecols[(p_, r)][:, 128 * s2:128 * s2 + 128],
                        in_=scrs[(s, r)]
                            .rearrange("p (k e) -> p k e", e=8)[:, :, 3:5],
                        func=AF.Exp, bias=0.0, scale=nrs[:, s:s + 1])

            xpgrp = xpgrps[g]
            mm8 = pairp.tile([64, 8], f32, tag=f"mm8_{g}", name=f"mm8_{g}")
            for q in range(2):
                p = 2 * g + q
                ps_rp = prp.tile([64, 256], f32, tag="prp")
                for r in range(4):
                    nc.tensor.matmul(ps_rp, csb["p025"][:, r, :], ecols[(p, r)],
                                     start=(r == 0), stop=(r == 3))
                rp_sb = pairp.tile([64, 256], f32, tag="rp_sb")
                nc.vector.tensor_copy(out=rp_sb, in_=ps_rp)
                rp64 = pairp.tile([64, 2, 64], f16, tag=f"rp64_{q}",
                                  name=f"rp64_{g}_{q}")
                v = rp_sb.rearrange("p (s k e) -> p s k e", s=2, e=2)
                nc.vector.tensor_tensor(out=rp64, in0=v[:, :, :, 0],
                                        in1=v[:, :, :, 1], op=ALU.add)
                rp64n = pairp.tile([64, 2, 64], f32, tag="rp64n")
                nc.vector.tensor_scalar_mul(out=rp64n, in0=rp64, scalar1=-1.0)
                nc.vector.tensor_reduce(out=mm8[:, 2 * q:2 * q + 2], in_=rp64,
                                        axis=mybir.AxisListType.X, op=ALU.max)
                nc.vector.tensor_reduce(out=mm8[:, 4 + 2 * q:6 + 2 * q],
                                        in_=rp64n,
                                        axis=mybir.AxisListType.X, op=ALU.max)
                for s2 in range(2):
                    eng = nc.sync if s2 == 0 else nc.scalar
                    eng.dma_start(
                        out=xpgrp[2 * q + s2:2 * q + s2 + 1, 0:66 * 66]
                            .rearrange("o (h w) -> o h w", w=66)[:, 1:65, 1:65],
                        in_=rp64[:, s2, :])

            ps_mm = psml.tile([8, 64], f32, tag="ps")
            nc.tensor.matmul(ps_mm, mm8, ident[0:64, 0:64], is_transpose=True)
            mnmx = pairp.tile([8, 1], f32, tag="mnmx")
            nc.vector.tensor_reduce(out=mnmx, in_=ps_mm,
                                    axis=mybir.AxisListType.X, op=ALU.max)
            ps_den = psml.tile([4, 1], f32, tag="ps")
            nc.tensor.matmul(ps_den, csb["m8sel"][:, 0:4], mnmx)
            ps_ngm = psml.tile([4, 1], f32, tag="ps")
            nc.tensor.matmul(ps_ngm, csb["m8sel"][:, 4:8], mnmx)
            sden = pairp.tile([4, 1], f32, tag="sden")
            rcp = pairp.tile([4, 1], f32, tag="rcp")
            ngm = pairp.tile([4, 1], f32, tag="ngm")
            nc.vector.tensor_scalar(out=sden, in0=ps_den, scalar1=1e-4,
                                    scalar2=None, op0=ALU.add, op1=ALU.bypass)
            nc.vector.reciprocal(out=rcp, in_=sden)
            nc.vector.tensor_copy(out=ngm, in_=ps_ngm)
            intv = xpgrp[:, 0:66 * 66].rearrange(
                "o (h w) -> o h w", w=66)[:, 1:65, 1:65]
            nc.vector.tensor_scalar(out=intv, in0=intv, scalar1=ngm,
                                    scalar2=rcp, op0=ALU.add, op1=ALU.mult)

            # L1 im2col bands (9 dy/dx-shifted copies).  Bands 0 and 32 are
            # 32-aligned so the DVE can build them directly (~free); the
            # other 7 spread over all three DMA queues (SBUF->SBUF DMA is
            # ~13GB/s serial per queue, and the Pool queue is idle here).
            imY = l1p.tile([36, 64 * 66], f16, tag=f"imY{g}", name=f"imY{g}")
            imYs.append(imY)
            engs3 = [nc.sync, nc.scalar, nc.gpsimd]
            i = 0
            for dx in range(3):
                for dy in range(3):
                    b = 12 * dx + 4 * dy
                    off = dy * 66 + dx
                    if b in (0, 32):
                        nc.vector.tensor_copy(
                            out=imY[b:b + 4, :],
                            in_=xpgrp[:, off:off + 64 * 66])
                    else:
                        engs3[i % 3].dma_start(
                            out=imY[b:b + 4, :],
                            in_=xpgrp[:, off:off + 64 * 66])
                        i += 1

        # ===== CNN L1 (K=36), both groups =====
        gl1s = []
        for g in range(2):
            imYv = imYs[g].rearrange("p (a b) -> p a b", b=66)
            gl1 = l1p.tile([128, 4096], f16, tag=f"gl1_{g}", name=f"gl1_{g}")
            gl1s.append(gl1)
            for cchunk in range(8):
                psL1 = pbig.tile([128, 512], f32, tag="pbig")
                nc.tensor.matmul(psL1, csb["c1imT"],
                                 imYv[:, 8 * cchunk:8 * cchunk + 8, 0:64])
                nc.scalar.activation(out=gl1[:, 512 * cchunk:512 * cchunk + 512],
                                     in_=psL1, func=AF.Gelu,
                                     bias=bn("cbn1b"), scale=bn("cbn1s"))

        # ===== pool1 into L2 band tiles, both groups =====
        for g in range(2):
            gl1 = gl1s[g]
            pm1 = l1p.tile([128, 64, 32], f16, tag=f"pm1_{g}", name=f"pm1_{g}")
            v1 = gl1.rearrange("p (h w e) -> p h w e", w=32, e=2)
            nc.vector.tensor_tensor(out=pm1, in0=v1[:, :, :, 0],
                                    in1=v1[:, :, :, 1], op=ALU.max)
            v2 = pm1.rearrange("p (h e) w -> p h e w", e=2)
            for q in range(2):
                bt = xl2b[(g, q)]
                nc.vector.tensor_tensor(
                    out=bt[0:64].rearrange("p (a b) -> p a b", b=34)
                        [:, 1:33, 1:33],
                    in0=v2[64 * q:64 * q + 64, :, 0, :],
                    in1=v2[64 * q:64 * q + 64, :, 1, :], op=ALU.max)
                # shifted duplicate band (tap pairs): rows 64-127 = <<1 col
                nc.vector.tensor_copy(out=bt[64:128, 0:34 * 34 - 1],
                                      in_=bt[0:64, 1:34 * 34])

        # ===== CNN L2..L4, interleaved so the PE never waits on pool chains
        def emit_l2(g, q):
            gl2 = l1p.tile([128, 1024], f16, tag=f"gl2_{q}")
            btv = xl2b[(g, q)].rearrange("p (a b) -> p a b", b=34)
            for cchunk in range(2):
                psL2 = pbig.tile([128, 512], f32, tag="pbig")
                h0 = 16 * cchunk
                for dy in range(3):
                    nc.tensor.matmul(
                        psL2, csb["w2p2"][:, dy, :],
                        btv[:, h0 + dy:h0 + dy + 16, 0:32],
                        start=(dy == 0), stop=False)
                for dy in range(3):
                    nc.tensor.matmul(
                        psL2, csb["w2s2"][:, dy, :],
                        btv[0:64, h0 + dy:h0 + dy + 16, 2:34],
                        start=False, stop=(dy == 2))
                nc.scalar.activation(
                    out=gl2[:, 512 * cchunk:512 * cchunk + 512], in_=psL2,
                    func=AF.Gelu, bias=bn("cbn2b"), scale=bn("cbn2s"))

            # maxpool 32x32 -> 16x16 into the per-sample L3 band tiles
            pm2 = l1p.tile([128, 32, 16], f16, tag=f"pm2_{q}")
            w1v = gl2.rearrange("p (h w e) -> p h w e", w=16, e=2)
            nc.vector.tensor_tensor(out=pm2, in0=w1v[:, :, :, 0],
                                    in1=w1v[:, :, :, 1], op=ALU.max)
            w2v = pm2.rearrange("p (h e) w -> p h e w", e=2)
            for s2 in range(2):
                bt3 = xl3b[(g, q, s2)]
                nc.vector.tensor_tensor(
                    out=bt3[0:64].rearrange("p (a b) -> p a b", b=18)
                        [:, 1:17, 1:17],
                    in0=w2v[64 * s2:64 * s2 + 64, :, 0, :],
                    in1=w2v[64 * s2:64 * s2 + 64, :, 1, :], op=ALU.max)
                nc.vector.tensor_copy(out=bt3[64:128, 0:18 * 18 - 1],
                                      in_=bt3[0:64, 1:18 * 18])

        def emit_l3(g, q, s2):
            sg = 2 * q + s2
            bt3v = xl3b[(g, q, s2)].rearrange("p (a b) -> p a b", b=18)
            psL3 = pbig.tile([128, 256], f32, tag="pbig")
            for dy in range(3):
                nc.tensor.matmul(psL3, csb["w3p"][:, dy, :],
                                 bt3v[:, dy:dy + 16, 0:16],
                                 start=(dy == 0), stop=False)
            for dy in range(3):
                nc.tensor.matmul(psL3, csb["w3s"][:, dy, :],
                                 bt3v[0:64, dy:dy + 16, 2:18],
                                 start=False, stop=(dy == 2))
            gl3 = l1p.tile([128, 256], f16, tag=f"gl3_{s2}")
            nc.scalar.activation(out=gl3, in_=psL3, func=AF.Gelu,
                                 bias=bn("cbn3b"), scale=bn("cbn3s"))
            # maxpool 16x16 -> 8x8 into l4in (10x10 padded)
            pm3 = l1p.tile([128, 16, 8], f16, tag=f"pm3_{s2}")
            u1 = gl3.rearrange("p (h w e) -> p h w e", w=8, e=2)
            nc.vector.tensor_tensor(out=pm3, in0=u1[:, :, :, 0],
                                    in1=u1[:, :, :, 1], op=ALU.max)
            u2 = pm3.rearrange("p (h e) w -> p h e w", e=2)
            nc.vector.tensor_tensor(
                out=l4ins[g].rearrange("p (s a b) -> p s a b", a=10, b=10)
                    [:, sg, 1:9, 1:9],
                in0=u2[:, :, 0, :], in1=u2[:, :, 1, :], op=ALU.max)

        def emit_l4(g):
            psL4 = pbig.tile([128, 256], f32, tag="pbig")
            xl4 = l4ins[g].rearrange("p (s a b) -> p s a b", a=10, b=10)
            for t in range(9):
                dy, dx = t // 3, t % 3
                nc.tensor.matmul(psL4, csb["cw4T"][:, t, :],
                                 xl4[:, :, dy:dy + 8, dx:dx + 8],
                                 start=(t == 0), stop=(t == 8))
            gl4 = l1p.tile([128, 256], f16, tag="gl4")
            nc.scalar.activation(out=gl4, in_=psL4, func=AF.Gelu,
                                 bias=bn("cbn4b"), scale=bn("cbn4s"))
            # avgpool 8x8 -> 4x4 (sum; 0.25 folded into fc1 weights)
            av1 = l1p.tile([128, 128], f16, tag="av1")
            a1 = gl4.rearrange("p (s h w e) -> p s h w e", s=4, w=4, e=2)
            nc.vector.tensor_tensor(
                out=av1.rearrange("p (s h w) -> p s h w", s=4, w=4),
                in0=a1[:, :, :, :, 0], in1=a1[:, :, :, :, 1], op=ALU.add)
            a2 = av1.rearrange("p (s h e w) -> p s h e w", s=4, e=2, w=4)
            nc.vector.tensor_tensor(out=fcin[:, 64 * g:64 * g + 64]
                                        .rearrange("p (s h w) -> p s h w", s=4, w=4),
                                    in0=a2[:, :, :, 0, :], in1=a2[:, :, :, 1, :],
                                    op=ALU.add)

        emit_l2(0, 0)
        emit_l2(0, 1)
        emit_l3(0, 0, 0)
        emit_l3(0, 0, 1)
        emit_l3(0, 1, 0)
        emit_l3(0, 1, 1)
        emit_l2(1, 0)
        emit_l2(1, 1)
        emit_l4(0)
        emit_l3(1, 0, 0)
        emit_l3(1, 0, 1)
        emit_l3(1, 1, 0)
        emit_l3(1, 1, 1)
        emit_l4(1)

        # ================= FC head =================
        ps_fc = prp.tile([8, 256], f32, tag="prp")
        fv = fcin.rearrange("p (s j) -> p s j", j=16)
        for j in range(16):
            nc.tensor.matmul(ps_fc, fv[:, :, j], csb["fc1wT"][:, j, :],
                             start=(j == 0), stop=False)
        nc.tensor.matmul(ps_fc, onesK1M8, csb["fc1brow"], start=False, stop=True)
        nc.scalar.activation(out=fch, in_=ps_fc, func=AF.Gelu)
        junk = sing.tile([8, 256], f32)
        res8 = sing.tile([8, 1], f32)
        nc.vector.scalar_tensor_tensor(out=junk, in0=fch, scalar=1.0,
                                       in1=csb["fc2wb"], op0=ALU.mult,
                                       op1=ALU.mult, accum_out=res8)
        res8b = sing.tile([8, 1], f32)
        nc.vector.tensor_tensor(out=res8b, in0=res8, in1=csb["fc2bias"],
                                op=ALU.add)
        nc.sync.dma_start(out=out, in_=res8b)


# ------------------------------------------------------------------ driver
_prog_cache = {}


def _get_program(debug=False):
    key = ("dbg" if debug else "main")
    if key not in _prog_cache:
        _prog_cache[key] = build_program(debug=debug)
    return _prog_cache[key]


def _im2col_x(xs):
    """(8, 8, 512) f32 -> (112, 4, 512) f16 conv1d-1 im2col.

    Partition row 16k + 8s2 + c, pair p, col t = xpad[2p + s2, c, t + k]
    (pad 3 left/right).
    """
    xp = np.zeros((SPC, 8, T + 6), np.float16)
    xp[:, :, 3:3 + T] = xs.astype(np.float16)
    im = np.empty((7, 2, 8, 4, T), np.float16)
    for k in range(7):
        v = xp[:, :, k:k + T].reshape(4, 2, 8, T)
        im[k] = v.transpose(1, 2, 0, 3)
    return np.ascontiguousarray(im.reshape(112, 4, T))


def _run(inputs, debug=False):
    x = np.ascontiguousarray(np.asarray(inputs["x"]), np.float32)
    assert x.shape == (64, 8, 512), x.shape
    consts = _pack_consts({k: np.asarray(v) for k, v in inputs.items()})
    nc = _get_program(debug=debug)
    in_maps = []
    for c in range(N_CORES):
        m = dict(consts)
        m["xim"] = _im2col_x(x[SPC * c:SPC * c + SPC])
        in_maps.append(m)
    return run_bass_kernel_spmd(nc, in_maps, list(range(N_CORES)))


def kernel(**inputs):
    res = _run(inputs, debug=False)
    return np.concatenate([res.results[c]["out"][:, 0] for c in range(N_CORES)])


# revision 59
# speedup vs baseline: 1.2483x; 1.0511x over previous
"""Trainium2 Bass kernel for nn_EndToEndRPModel.

Pipeline per sample: conv1d stack (8ch,T=512 -> 6ch) -> pairwise-distance
soft recurrence plot (512x512) -> bilinear resize to 64x64 (exact 2x2 mean
of a strided 128x128 subgrid since scale=8) -> min-max norm -> small CNN ->
FC head -> scalar.

Sharding: pure data parallel, 8 samples per core on 8 cores.

Key implementation notes:
 - all heavy matmuls run in fp16 (1 cyc/col); d2 = sq_i + sq_j - 2*gram via
   ONE augmented fp16 matmul per 128-row tile, 4 samples packed into
   disjoint PE row quadrants via tile_position.
 - d2 diagonal forced to dist=1e-3 with gpsimd.affine_select restricted to
   the 16-col diagonal window of each row tile.
 - bilinear(512->64) == 0.25 * 2x2-sum over rows/cols {8j+3, 8j+4}; row
   selection+0.25 folded into a pooling matmul, col selection into the exp.
 - conv1d-2 / CNN L2 / CNN L3 run as tap-pair matmuls (K=128: two
   column-shifted copies of the input stacked in the partition dim),
   plus one single-tap matmul for the odd tap column.
 - CNN L1 runs as K=36 matmuls with all 9 taps baked into 9 dy/dx-shifted
   partition bands of the im2col tile (built with cheap vector copies).
 - sigma chain batched per group (one reduce + 2 tiny matmuls for 4
   samples).
 - narrow constants ship transposed (few wide DMA descriptors) and are
   transposed back on the PE at startup; identities/memsets are emitted
   before any Pool-queue DMA so nothing blocks them.
 - all BN affines folded into the Gelu activation's per-partition
   scale/bias; avgpool's 0.25 folded into the FC1 weights.
"""
import sys

sys.path.insert(0, "/opt/trn_rl_repo")

import numpy as np

import concourse.bacc as bacc
import concourse.tile as tile
from concourse import mybir
from concourse.bass_utils import run_bass_kernel_spmd
from concourse.masks import make_identity

f32 = mybir.dt.float32
f32r = mybir.dt.float32r
f16 = mybir.dt.float16
AF = mybir.ActivationFunctionType
ALU = mybir.AluOpType

N_CORES = 8
SPC = 8          # samples per core
T = 512
BN_KAPPA = 1.0 / np.sqrt(1.0 + 1e-5)


# ---------------------------------------------------------------- host-side
def _pack_consts(inp):
    """Pack all weights into the exact SBUF layouts the kernel uses."""
    c16 = {}
    c32 = {}
    w1 = inp["w1"]; w2 = inp["w2"]; w3 = inp["w3"]

    # conv1d-1 im2col weights: rows 16k + 8s2 + ch, cols 32s2 + o
    # shipped transposed [64, 112] and PE-transposed on chip.
    w1imT = np.zeros((112, 64), np.float32)
    for k in range(7):
        for s2 in range(2):
            w1imT[16 * k + 8 * s2:16 * k + 8 * s2 + 8, 32 * s2:32 * s2 + 32] = \
                w1[:, :, k].T
    c16["w1imTT"] = np.ascontiguousarray(w1imT.T)

    # conv1d-2 tap-pair weights: rounds (0,1), (2,3) are [128, 128]
    # (rows 64b + 32s2 + ch for band b in {tap k, tap k+1}); round 4 is
    # [64, 128] single-tap.  Shipped as one [128, 3, 128] tensor.
    w2p = np.zeros((128, 3, 128), np.float32)
    for rnd, k0 in enumerate((0, 2)):
        for b in range(2):
            for s2 in range(2):
                w2p[64 * b + 32 * s2:64 * b + 32 * s2 + 32, rnd,
                    64 * s2:64 * s2 + 64] = w2[:, :, k0 + b].T
    for s2 in range(2):
        w2p[32 * s2:32 * s2 + 32, 2, 64 * s2:64 * s2 + 64] = w2[:, :, 4].T
    c16["w2p"] = w2p

    # conv1d-3 taps: (128, 3, 12): rows 64s2+ch, cols 6s2+d
    # shipped transposed [36, 128] (rows 12k + 6s2 + d) and PE-transposed,
    # then scattered into even/odd-pair lhsT tiles on chip (z output lands
    # at 32-aligned psum rows 32sg+d so DVE can copy it).
    w3T = np.zeros((128, 3, 12), np.float32)
    for k in range(3):
        for s2 in range(2):
            w3T[64 * s2:64 * s2 + 64, k, 6 * s2:6 * s2 + 6] = w3[:, :, k].T
    c16["w3TT"] = np.ascontiguousarray(w3T.reshape(128, 36).T)

    # sq selector: rows 32sg + d -> col s2 (even/odd pair via 64-row halves)
    sqsel = np.zeros((128, 2), np.float32)
    for h in range(2):
        for s2 in range(2):
            sqsel[64 * h + 32 * s2:64 * h + 32 * s2 + 6, s2] = 1.0
    c32["sqselT"] = np.ascontiguousarray(sqsel.T)

    # pooling matrix for rp row-pairs: p025[p, r, j] = 0.25 if 128r+p in {8j+3, 8j+4}
    p025 = np.zeros((128, 4, 64), np.float32)
    for r in range(4):
        for p in range(128):
            i = 128 * r + p
            if i % 8 in (3, 4):
                j = (i - 3) // 8 if i % 8 == 3 else (i - 4) // 8
                if 0 <= j < 64:
                    p025[p, r, j] = 0.25
    c16["p025"] = p025

    # min-max combiner: mnmx8 rows = [mx0..mx3, -mn0..-mn3]
    m8 = np.zeros((8, 8), np.float32)
    for s in range(4):
        m8[s, s] = m8[4 + s, s] = 1.0    # den_s = mx_s + (-mn_s)
        m8[4 + s, 4 + s] = 1.0           # negmn_s
    c32["m8sel"] = m8

    # 2D conv weights
    c1 = inp["c1"]; c2 = inp["c2"]; c3 = inp["c3"]; c4 = inp["c4"]
    # L1: K=36 im2col, rows 12dx + 4dy + s, cols 32s + o
    c1imT = np.zeros((36, 128), np.float32)
    for s in range(4):
        for dy in range(3):
            for dx in range(3):
                c1imT[12 * dx + 4 * dy + s, 32 * s:32 * s + 32] = c1[:, 0, dy, dx]
    c16["c1imT"] = c1imT

    # L2 tap-pair weights: bands [s0, s1, s0<<1col, s1<<1col] x 32ch
    w2p2 = np.zeros((128, 3, 128), np.float32)
    w2s2 = np.zeros((64, 3, 128), np.float32)
    for dy in range(3):
        for s2 in range(2):
            w2p2[32 * s2:32 * s2 + 32, dy, 64 * s2:64 * s2 + 64] = \
                c2[:, :, dy, 0].T
            w2p2[64 + 32 * s2:64 + 32 * s2 + 32, dy, 64 * s2:64 * s2 + 64] = \
                c2[:, :, dy, 1].T
            w2s2[32 * s2:32 * s2 + 32, dy, 64 * s2:64 * s2 + 64] = \
                c2[:, :, dy, 2].T
    c16["w2p2"] = w2p2
    c16["w2s2"] = w2s2

    # L3 tap-pair weights: bands [64ch, 64ch<<1col]
    w3p = np.zeros((128, 3, 128), np.float32)
    w3s = np.zeros((64, 3, 128), np.float32)
    for dy in range(3):
        w3p[0:64, dy, :] = c3[:, :, dy, 0].T
        w3p[64:128, dy, :] = c3[:, :, dy, 1].T
        w3s[:, dy, :] = c3[:, :, dy, 2].T
    c16["w3p"] = w3p
    c16["w3s"] = w3s

    cw4T = np.zeros((128, 9, 128), np.float32)
    for t in range(9):
        dy, dx = t // 3, t % 3
        cw4T[:, t, :] = c4[:, :, dy, dx].T
    c16["cw4T"] = cw4T

    # FC1 weights: (128, 16, 256), 0.25 avgpool folded in
    fc1_w = np.asarray(inp["fc1_w"], np.float32)        # (256, 2048)
    c16["fc1wT"] = 0.25 * np.ascontiguousarray(
        fc1_w.reshape(256, 128, 16).transpose(1, 2, 0))
    c16["fc1brow"] = inp["fc1_b"].reshape(1, 256).astype(np.float32)
    c32["fc2wb"] = np.broadcast_to(
        inp["fc2_w"].reshape(1, 256), (8, 256)).astype(np.float32).copy()
    c32["fc2bias"] = np.full(
        (8, 1), float(np.asarray(inp["fc2_b"]).reshape(-1)[0]), np.float32)

    # BN scale/bias vectors, one [16, 128] f32 pack shipped transposed.
    def rep(v, reps):
        return np.tile(np.asarray(v, np.float32), reps)
    bnT = np.zeros((16, 128), np.float32)
    bnT[0] = rep(inp["g1"] * BN_KAPPA, 4); bnT[1] = rep(inp["b1"], 4)
    bnT[2] = rep(inp["g2"] * BN_KAPPA, 2); bnT[3] = rep(inp["b2"], 2)
    bnT[4] = rep(inp["cg1"] * BN_KAPPA, 4); bnT[5] = rep(inp["cb1"], 4)
    bnT[6] = rep(inp["cg2"] * BN_KAPPA, 2); bnT[7] = rep(inp["cb2"], 2)
    bnT[8] = inp["cg3"] * BN_KAPPA; bnT[9] = inp["cb3"]
    bnT[10] = inp["cg4"] * BN_KAPPA; bnT[11] = inp["cb4"]
    c32["bnT"] = bnT

    out = {k: np.ascontiguousarray(v, np.float16) for k, v in c16.items()}
    out.update({k: np.ascontiguousarray(v, np.float32) for k, v in c32.items()})
    return out


# ------------------------------------------------------------- bass program
_C16_SHAPES = {
    "w1imTT": (64, 112), "w2p": (128, 3, 128), "w3TT": (36, 128),
    "p025": (128, 4, 64), "c1imT": (36, 128), "w2p2": (128, 3, 128),
    "w2s2": (64, 3, 128), "w3p": (128, 3, 128), "w3s": (64, 3, 128),
    "cw4T": (128, 9, 128), "fc1wT": (128, 16, 256), "fc1brow": (1, 256),
}
_C32_SHAPES = {
    "sqselT": (2, 128), "m8sel": (8, 8), "fc2wb": (8, 256), "fc2bias": (8, 1),
    "bnT": (16, 128),
}

BN_COL = {"bn1s": 0, "bn1b": 1, "bn2s": 2, "bn2b": 3, "cbn1s": 4, "cbn1b": 5,
          "cbn2s": 6, "cbn2b": 7, "cbn3s": 8, "cbn3b": 9, "cbn4s": 10,
          "cbn4b": 11}


def build_program(debug=False):
    nc = bacc.Bacc("TRN2", target_bir_lowering=False, debug=False,
                   num_devices=N_CORES)
    xim = nc.dram_tensor("xim", [112, 4, T], f16, kind="ExternalInput").ap()
    dram = {n: nc.dram_tensor(n, list(s), f16, kind="ExternalInput").ap()
            for n, s in _C16_SHAPES.items()}
    dram.update({n: nc.dram_tensor(n, list(s), f32, kind="ExternalInput").ap()
                 for n, s in _C32_SHAPES.items()})
    out = nc.dram_tensor("out", [SPC, 1], f32, kind="ExternalOutput").ap()

    with tile.TileContext(nc) as tc:
        _emit(tc, nc, xim, dram, out)
    nc.compile()
    return nc


def _emit(tc, nc, xim, dram, out):
    from contextlib import ExitStack
    ctx = ExitStack()
    with ctx:
        cpool = ctx.enter_context(tc.tile_pool(name="consts", bufs=1))
        sing = ctx.enter_context(tc.tile_pool(name="sing", bufs=1))
        c1p = ctx.enter_context(tc.tile_pool(name="conv1", bufs=3))
        dstp = ctx.enter_context(tc.tile_pool(name="dist", bufs=3))
        pairp = ctx.enter_context(tc.tile_pool(name="pairs", bufs=2))
        ecolp = ctx.enter_context(tc.tile_pool(name="ecols", bufs=1))
        grpp = ctx.enter_context(tc.tile_pool(name="grp", bufs=1))
        l1p = ctx.enter_context(tc.tile_pool(name="lcnn", bufs=1))
        pbig = ctx.enter_context(tc.tile_pool(name="pbig", bufs=6, space="PSUM"))
        prp = ctx.enter_context(tc.tile_pool(name="prp", bufs=1, space="PSUM"))
        psml = ctx.enter_context(tc.tile_pool(name="psml", bufs=1, space="PSUM"))

        # ------------- persistent tiles (allocated before anything runs)
        zaug_m = [grpp.tile([128, T], f16, tag=f"zaug_m{g}", name=f"zaug_m{g}")
                  for g in range(2)]
        zaug_s = [grpp.tile([128, T], f16, tag=f"zaug_s{g}", name=f"zaug_s{g}")
                  for g in range(2)]
        # 2 extra cols so the (dy=2, dx=2) L1 im2col band read stays in range
        xpgrps = [grpp.tile([4, 66 * 66 + 2], f16, tag=f"xpg{g}",
                            name=f"xpg{g}") for g in range(2)]
        xl2b = {(g, q): l1p.tile([128, 34 * 34], f16, tag=f"xl2b_{g}_{q}",
                                 name=f"xl2b_{g}_{q}")
                for g in range(2) for q in range(2)}
        xl3b = {(g, q, s2): l1p.tile([128, 18 * 18], f16,
                                     tag=f"xl3b_{g}_{q}_{s2}",
                                     name=f"xl3b_{g}_{q}_{s2}")
                for g in range(2) for q in range(2) for s2 in range(2)}
        zsq = sing.tile([128, T], f16)
        sqr_sb = sing.tile([128, T], f16)     # pair p sq rows at 32p, 32p+1
        rs = sing.tile([128, 8, 4], f32)       # sqrt row-sums per (s, r)
        rrt = sing.tile([128, 8], f32)
        nrs = sing.tile([128, 8], f32)         # -1/sigma broadcast per sample
        fcin = sing.tile([128, 128], f16)
        fch = sing.tile([8, 256], f32)

        # ------------- setup on the (otherwise idle) gpsimd queue so the
        # vector queue stays free for the startup const copies.  Order:
        # identities (gate the PE transposes), then tiles needed by conv1d,
        # then the late-phase tiles.  zaug ones-rows are filled as [8, T]
        # 32-aligned strips: value!=0 memsets are ~10x slower than 0.0 and
        # cost scales with rows, and later z/-2z/sq writes overwrite 7 of 8.
        ident = cpool.tile([128, 128], f32)
        make_identity(nc, ident)
        identh = cpool.tile([64, 64], f16)
        make_identity(nc, identh)
        ones128x1 = cpool.tile([128, 1], f32)
        nc.gpsimd.memset(ones128x1, 1.0)
        ones1x128 = cpool.tile([1, 128], f32)
        nc.gpsimd.memset(ones1x128, 1.0)
        # conv1d-3 lhsT (cols 0:6 / 32:38 so both samples' z lands at
        # 32-aligned psum rows for every pair)
        w3T38 = cpool.tile([128, 3, 38], f16, name="w3T38")
        nc.gpsimd.memset(w3T38, 0.0)
        for g in range(2):
            for sg in range(4):
                nc.gpsimd.memset(zaug_m[g][32 * sg:32 * sg + 8, :], 1.0)
                nc.gpsimd.memset(zaug_s[g][32 * sg:32 * sg + 8, :], 1.0)
        nc.gpsimd.memset(zsq, 0.0)
        for g in range(2):
            nc.gpsimd.memset(xpgrps[g], 0.0)
        for t in xl2b.values():
            nc.gpsimd.memset(t, 0.0)
        for t in xl3b.values():
            nc.gpsimd.memset(t, 0.0)
        l4ins = [l1p.tile([128, 400], f16, tag=f"l4in{g}", name=f"l4in{g}")
                 for g in range(2)]
        nc.gpsimd.memset(l4ins[0], 0.0)
        nc.gpsimd.memset(l4ins[1], 0.0)

        # ---------------- constants into SBUF (sync + scalar queues)
        csb = {}

        def ctile(n):
            shape = _C16_SHAPES.get(n) or _C32_SHAPES[n]
            t = cpool.tile(list(shape), f16 if n in _C16_SHAPES else f32,
                           name="c_" + n, tag="c_" + n)
            csb[n] = t
            return t

        # Stage 0: first-matmul critical
        t_w1 = ctile("w1imTT")
        nc.sync.dma_start(out=t_w1[0:32], in_=dram["w1imTT"][0:32])
        nc.scalar.dma_start(out=t_w1[32:64], in_=dram["w1imTT"][32:64])
        imall = c1p.tile([112, 4, T], f16, tag="imall", bufs=1, name="imall")
        nc.sync.dma_start(out=imall[0:56, 0], in_=xim[0:56, 0])
        nc.scalar.dma_start(out=imall[56:112, 0], in_=xim[56:112, 0])

        # Stage 1: phase-1 weights + remaining input pairs
        t_bnT = ctile("bnT")
        nc.scalar.dma_start(out=t_bnT, in_=dram["bnT"])
        t_w2p = ctile("w2p")
        nc.sync.dma_start(out=t_w2p[0:64], in_=dram["w2p"][0:64])
        nc.scalar.dma_start(out=t_w2p[64:128], in_=dram["w2p"][64:128])
        t_w3 = ctile("w3TT")
        nc.sync.dma_start(out=t_w3, in_=dram["w3TT"])
        t_sq = ctile("sqselT")
        nc.scalar.dma_start(out=t_sq, in_=dram["sqselT"])
        # pairs 1-3 fully on sync: their 168 descriptors would otherwise
        # queue ahead of the first gelu dispatch on the Act queue
        for p in range(1, 4):
            nc.sync.dma_start(out=imall[0:56, p], in_=xim[0:56, p])
            nc.sync.dma_start(out=imall[56:112, p], in_=xim[56:112, p])

        # Stage 2a: small/mid weights.  The Act queue gets only 16
        # descriptors here — gelu dispatch queues behind everything ahead
        # of it on the Act queue, so keep that queue on a DMA diet.
        QS = [nc.sync, nc.scalar]
        for n in ["m8sel", "fc1brow", "p025", "c1imT"]:
            t = ctile(n)
            nc.sync.dma_start(out=t, in_=dram[n])
        for n in ["fc2bias", "fc2wb"]:
            t = ctile(n)
            nc.scalar.dma_start(out=t, in_=dram[n])

        onesK1M8 = cpool.tile([1, 8], f16)
        nc.gpsimd.memset(onesK1M8, 1.0)

        # --------- on-chip transposes of narrow consts (also warms the PE)
        ps_t1 = psml.tile([112, 64], f16, tag="ps")
        nc.tensor.matmul(ps_t1, csb["w1imTT"], identh, is_transpose=True)
        w1imT = cpool.tile([112, 64], f16, name="w1imT")
        nc.vector.tensor_copy(out=w1imT, in_=ps_t1)

        ps_t2 = psml.tile([128, 36], f16, tag="ps")
        nc.tensor.matmul(ps_t2, csb["w3TT"], identh[0:36, 0:36],
                         is_transpose=True)
        w3Tsb = cpool.tile([128, 36], f16, name="w3Tsb")
        nc.vector.tensor_copy(out=w3Tsb, in_=ps_t2)
        w3v = w3Tsb.rearrange("p (k sd) -> p k sd", sd=12)
        for k in range(3):
            nc.vector.tensor_copy(out=w3T38[:, k, 0:6], in_=w3v[:, k, 0:6])
            nc.vector.tensor_copy(out=w3T38[:, k, 32:38], in_=w3v[:, k, 6:12])

        ps_t3 = psml.tile([128, 16], f32, tag="ps")
        nc.tensor.matmul(ps_t3, csb["bnT"], ident[0:16, 0:16],
                         is_transpose=True)
        bnsb = cpool.tile([128, 16], f32, name="bnsb")
        nc.vector.tensor_copy(out=bnsb, in_=ps_t3)

        ps_t4 = psml.tile([128, 2], f32, tag="ps")
        nc.tensor.matmul(ps_t4, csb["sqselT"], ident[0:2, 0:2],
                         is_transpose=True)
        sqsel = cpool.tile([128, 2], f16, name="sqsel")
        nc.vector.tensor_copy(out=sqsel, in_=ps_t4)

        def bn(name, n=128):
            return bnsb[0:n, BN_COL[name]:BN_COL[name] + 1]

        # ================= PHASE 1: conv1d, batch-emitted by half =========
        # z and sq rows are written straight into the group-major augmented
        # tiles (zm rows 32sg+[0..5] = z, +6 = ones, +7 = sq; zs = -2z / sq
        # at +6 / ones at +7).
        for half in range(2):
            pairs = (2 * half, 2 * half + 1)
            ps1s = {}
            for p in pairs:
                ps1 = pbig.tile([64, T], f32, tag="pbig")
                nc.tensor.matmul(ps1, w1imT, imall[:, p])
                ps1s[p] = ps1
            h1s = {}
            for p in pairs:
                # h1pad2: rows 0-63 = gelu(bn(ps1)) at cols 2..514 (pad 2
                # each side); rows 64-127 = same shifted left 1 col.
                h1pad2 = c1p.tile([128, T + 4], f16, tag="h1pad2")
                if p < 3:
                    nc.vector.memset(h1pad2[:, 0:2], 0.0)
                    nc.vector.memset(h1pad2[:, T + 1:T + 4], 0.0)
                nc.scalar.activation(out=h1pad2[0:64, 2:2 + T], in_=ps1s[p],
                                     func=AF.Gelu, bias=bn("bn1b", 64),
                                     scale=bn("bn1s", 64))
                nc.vector.tensor_copy(out=h1pad2[64:128, 1:1 + T],
                                      in_=h1pad2[0:64, 2:2 + T])
                h1s[p] = h1pad2
            ps2s = {}
            for p in pairs:
                h1pad2 = h1s[p]
                ps2 = pbig.tile([128, T], f32, tag="pbig")
                nc.tensor.matmul(ps2, csb["w2p"][:, 0, :], h1pad2[:, 0:T],
                                 start=True, stop=False)
                nc.tensor.matmul(ps2, csb["w2p"][:, 1, :], h1pad2[:, 2:2 + T],
                                 start=False, stop=False)
                nc.tensor.matmul(ps2, csb["w2p"][0:64, 2, :],
                                 h1pad2[0:64, 4:4 + T],
                                 start=False, stop=True)
                ps2s[p] = ps2
            h2s = {}
            for p in pairs:
                h2pad = c1p.tile([128, T + 2], f16, tag="h2pad")
                if p < 3:
                    nc.vector.memset(h2pad[:, 0:1], 0.0)
                    nc.vector.memset(h2pad[:, T + 1:T + 2], 0.0)
                nc.scalar.activation(out=h2pad[:, 1:1 + T], in_=ps2s[p],
                                     func=AF.Gelu, bias=bn("bn2b"),
                                     scale=bn("bn2s"))
                h2s[p] = h2pad
            ps3s = {}
            for p in pairs:
                ps3 = pbig.tile([38, T], f32, tag="pbig")
                for k in range(3):
                    nc.tensor.matmul(ps3, w3T38[:, k, :], h2s[p][:, k:k + T],
                                     start=(k == 0), stop=(k == 2))
                ps3s[p] = ps3
            for p in pairs:
                g, sgb, h = p // 2, 2 * (p % 2), p % 2
                ps3 = ps3s[p]
                # z / -2z / z^2; psum rows {0:6, 32:38} -> zaug rows 32sg+d
                for s2 in range(2):
                    r0 = 32 * (sgb + s2)
                    nc.vector.tensor_copy(out=zaug_m[g][r0:r0 + 6, :],
                                          in_=ps3[32 * s2:32 * s2 + 6, :])
                    nc.vector.tensor_scalar_mul(out=zaug_s[g][r0:r0 + 6, :],
                                                in0=ps3[32 * s2:32 * s2 + 6, :],
                                                scalar1=-2.0)
                    nc.vector.tensor_mul(out=zsq[64 * h + 32 * s2:
                                                 64 * h + 32 * s2 + 6, :],
                                         in0=zaug_m[g][r0:r0 + 6, :],
                                         in1=zaug_m[g][r0:r0 + 6, :])
                ps_sq = psml.tile([2, T], f32, tag="ps")
                nc.tensor.matmul(ps_sq, sqsel[64 * h:64 * h + 38, :],
                                 zsq[64 * h:64 * h + 38, :],
                                 tile_position=(64 * h, 0))
                nc.vector.tensor_copy(out=sqr_sb[32 * p:32 * p + 2, :],
                                      in_=ps_sq)
                for s2 in range(2):
                    sg = sgb + s2
                    eng = QS[s2 % 2]
                    eng.dma_start(
                        out=zaug_m[g][32 * sg + 7:32 * sg + 8, :],
                        in_=sqr_sb[32 * p + s2:32 * p + s2 + 1, :])
                    eng.dma_start(
                        out=zaug_s[g][32 * sg + 6:32 * sg + 7, :],
                        in_=sqr_sb[32 * p + s2:32 * p + s2 + 1, :])

        # Stage 2b: bulky CNN/FC weights, emitted after conv1d and entirely
        # on sync: the Act queue must stay empty between the conv gelus and
        # the dist sqrts (H-regression lesson), and sync's later DMAs here
        # (sq-rows above: 1 descriptor each; scatters: needed ~75us) still
        # dispatch in time behind these ~640 descriptors.
        for n in ["w2p2", "w2s2", "w3p", "w3s", "cw4T", "fc1wT"]:
            t = ctile(n)
            nc.sync.dma_start(out=t, in_=dram[n])

        # ===== dist matmul + clamp + sqrt + diag-fix, both groups =====
        # (all sqrts emitted before any exp so the Scalar queue never
        # head-of-line-blocks group 1's sqrts behind group 0's exps)
        ecols = {}   # (pair, r) -> (128, 256) f16
        scrs = {}
        for g in range(2):
            for sg in range(4):
                s = 4 * g + sg
                for r in range(4):
                    psd = pbig.tile([128, T], f32, tag="pbig")
                    nc.tensor.matmul(psd,
                                     zaug_s[g][32 * sg:32 * sg + 8,
                                               128 * r:128 * r + 128],
                                     zaug_m[g][32 * sg:32 * sg + 8, :],
                                     tile_position=(32 * sg, 0))
                    dmax = dstp.tile([128, T], f16, tag="dmax", bufs=6)
                    nc.vector.tensor_scalar(out=dmax, in0=psd, scalar1=0.0,
                                            scalar2=1e-6, op0=ALU.max,
                                            op1=ALU.add)
                    scr = dstp.tile([128, T], f16, tag=f"scr_{s}_{r}",
                                    bufs=1, name=f"scr_{s}_{r}")
                    nc.scalar.activation(out=scr, in_=dmax, func=AF.Sqrt,
                                         bias=0.0, scale=1.0,
                                         accum_out=rs[:, s, r:r + 1])
                    # exact diag dist = 1e-3; the diagonal lives at
                    # k in [16r, 16r+16) where col {8k+3+e} == row p + 128r.
                    nc.gpsimd.affine_select(
                        out=scr.rearrange("p (k e) -> p k e", e=8)
                            [:, 16 * r:16 * r + 16, 3:5],
                        in_=scr.rearrange("p (k e) -> p k e", e=8)
                            [:, 16 * r:16 * r + 16, 3:5],
                        compare_op=ALU.not_equal, fill=1e-3,
                        base=-3, pattern=[[-8, 16], [-1, 2]],
                        channel_multiplier=1)
                    scrs[(s, r)] = scr

        # ===== per group: sigma -> exp -> rp pooling -> norm -> imY =====
        imYs = []
        for g in range(2):
            nc.vector.tensor_reduce(out=rrt[:, 4 * g:4 * g + 4],
                                    in_=rs[:, 4 * g:4 * g + 4, :],
                                    axis=mybir.AxisListType.X, op=ALU.add)
            ps_s4 = psml.tile([1, 4], f32, tag="ps")
            nc.tensor.matmul(ps_s4, ones128x1, rrt[:, 4 * g:4 * g + 4])
            sgr = dstp.tile([1, 4], f32, tag="sgr")
            nc.vector.tensor_scalar(out=sgr, in0=ps_s4,
                                    scalar1=-1.0 / (T * T), scalar2=-1e-4,
                                    op0=ALU.mult, op1=ALU.add)
            nc.vector.reciprocal(out=sgr, in_=sgr)
            ps_n4 = psml.tile([128, 4], f32, tag="ps")
            nc.tensor.matmul(ps_n4, ones1x128, sgr)
            nc.vector.tensor_copy(out=nrs[:, 4 * g:4 * g + 4], in_=ps_n4)

            # exp on the strided subgrid columns {8k+3, 8k+4}
            for sg in range(4):
                s = 4 * g + sg
                p_, s2 = divmod(s, 2)
                for r in range(4):
                    if (p_, r) not in ecols:
                        ecols[(p_, r)] = ecolp.tile(
                            [128, 256], f16, tag=f"ecols_{p_}_{r}",
                            name=f"ecols_{p_}_{r}")
                    nc.scalar.activation(
                        out=ecols[(p_, r)][:, 128 * s2:128 * s2 + 128],
                        in_=scrs[(s, r)]
                            .rearrange("p (k e) -> p k e", e=8)[:, :, 3:5],
                        func=AF.Exp, bias=0.0, scale=nrs[:, s:s + 1])

            xpgrp = xpgrps[g]
            mm8 = pairp.tile([64, 8], f32, tag=f"mm8_{g}", name=f"mm8_{g}")
            for q in range(2):
                p = 2 * g + q
                ps_rp = prp.tile([64, 256], f32, tag="prp")
                for r in range(4):
                    nc.tensor.matmul(ps_rp, csb["p025"][:, r, :], ecols[(p, r)],
                                     start=(r == 0), stop=(r == 3))
                rp_sb = pairp.tile([64, 256], f32, tag="rp_sb")
                nc.vector.tensor_copy(out=rp_sb, in_=ps_rp)
                rp64 = pairp.tile([64, 2, 64], f16, tag=f"rp64_{q}",
                                  name=f"rp64_{g}_{q}")
                v = rp_sb.rearrange("p (s k e) -> p s k e", s=2, e=2)
                nc.vector.tensor_tensor(out=rp64, in0=v[:, :, :, 0],
                                        in1=v[:, :, :, 1], op=ALU.add)
                rp64n = pairp.tile([64, 2, 64], f32, tag="rp64n")
                nc.vector.tensor_scalar_mul(out=rp64n, in0=rp64, scalar1=-1.0)
                nc.vector.tensor_reduce(out=mm8[:, 2 * q:2 * q + 2], in_=rp64,
                                        axis=mybir.AxisListType.X, op=ALU.max)
                nc.vector.tensor_reduce(out=mm8[:, 4 + 2 * q:6 + 2 * q],
                                        in_=rp64n,
                                        axis=mybir.AxisListType.X, op=ALU.max)
                for s2 in range(2):
                    eng = nc.sync if s2 == 0 else nc.scalar
                    eng.dma_start(
                        out=xpgrp[2 * q + s2:2 * q + s2 + 1, 0:66 * 66]
                            .rearrange("o (h w) -> o h w", w=66)[:, 1:65, 1:65],
                        in_=rp64[:, s2, :])

            ps_mm = psml.tile([8, 64], f32, tag="ps")
            nc.tensor.matmul(ps_mm, mm8, ident[0:64, 0:64], is_transpose=True)
            mnmx = pairp.tile([8, 1], f32, tag="mnmx")
            nc.vector.tensor_reduce(out=mnmx, in_=ps_mm,
                                    axis=mybir.AxisListType.X, op=ALU.max)
            ps_den = psml.tile([4, 1], f32, tag="ps")
            nc.tensor.matmul(ps_den, csb["m8sel"][:, 0:4], mnmx)
            ps_ngm = psml.tile([4, 1], f32, tag="ps")
            nc.tensor.matmul(ps_ngm, csb["m8sel"][:, 4:8], mnmx)
            sden = pairp.tile([4, 1], f32, tag="sden")
            rcp = pairp.tile([4, 1], f32, tag="rcp")
            ngm = pairp.tile([4, 1], f32, tag="ngm")
            nc.vector.tensor_scalar(out=sden, in0=ps_den, scalar1=1e-4,
                                    scalar2=None, op0=ALU.add, op1=ALU.bypass)
            nc.vector.reciprocal(out=rcp, in_=sden)
            nc.vector.tensor_copy(out=ngm, in_=ps_ngm)
            intv = xpgrp[:, 0:66 * 66].rearrange(
                "o (h w) -> o h w", w=66)[:, 1:65, 1:65]
            nc.vector.tensor_scalar(out=intv, in0=intv, scalar1=ngm,
                                    scalar2=rcp, op0=ALU.add, op1=ALU.mult)

            # L1 im2col bands (9 dy/dx-shifted copies).  Bands 0 and 32 are
            # 32-aligned so the DVE can build them directly (~free); the
            # other 7 spread over all three DMA queues (SBUF->SBUF DMA is
            # ~13GB/s serial per queue, and the Pool queue is idle here).
            imY = l1p.tile([36, 64 * 66], f16, tag=f"imY{g}", name=f"imY{g}")
            imYs.append(imY)
            engs3 = [nc.sync, nc.scalar, nc.gpsimd]
            i = 0
            for dx in range(3):
                for dy in range(3):
                    b = 12 * dx + 4 * dy
                    off = dy * 66 + dx
                    if b in (0, 32):
                        nc.vector.tensor_copy(
                            out=imY[b:b + 4, :],
                            in_=xpgrp[:, off:off + 64 * 66])
                    else:
                        engs3[i % 3].dma_start(
                            out=imY[b:b + 4, :],
                            in_=xpgrp[:, off:off + 64 * 66])
                        i += 1

        # ===== CNN L1 (K=36), both groups =====
        gl1s = []
        for g in range(2):
            imYv = imYs[g].rearrange("p (a b) -> p a b", b=66)
            gl1 = l1p.tile([128, 4096], f16, tag=f"gl1_{g}", name=f"gl1_{g}")
            gl1s.append(gl1)
            for cchunk in range(8):
                psL1 = pbig.tile([128, 512], f32, tag="pbig")
                nc.tensor.matmul(psL1, csb["c1imT"],
                                 imYv[:, 8 * cchunk:8 * cchunk + 8, 0:64])
                nc.scalar.activation(out=gl1[:, 512 * cchunk:512 * cchunk + 512],
                                     in_=psL1, func=AF.Gelu,
                                     bias=bn("cbn1b"), scale=bn("cbn1s"))

        # ===== pool1 into L2 band tiles, both groups =====
        for g in range(2):
            gl1 = gl1s[g]
            pm1 = l1p.tile([128, 64, 32], f16, tag=f"pm1_{g}", name=f"pm1_{g}")
            v1 = gl1.rearrange("p (h w e) -> p h w e", w=32, e=2)
            nc.vector.tensor_tensor(out=pm1, in0=v1[:, :, :, 0],
                                    in1=v1[:, :, :, 1], op=ALU.max)
            v2 = pm1.rearrange("p (h e) w -> p h e w", e=2)
            for q in range(2):
                bt = xl2b[(g, q)]
                nc.vector.tensor_tensor(
                    out=bt[0:64].rearrange("p (a b) -> p a b", b=34)
                        [:, 1:33, 1:33],
                    in0=v2[64 * q:64 * q + 64, :, 0, :],
                    in1=v2[64 * q:64 * q + 64, :, 1, :], op=ALU.max)
                # shifted duplicate band (tap pairs): rows 64-127 = <<1 col
                nc.vector.tensor_copy(out=bt[64:128, 0:34 * 34 - 1],
                                      in_=bt[0:64, 1:34 * 34])

        # ===== CNN L2..L4, interleaved so the PE never waits on pool chains
        def emit_l2(g, q):
            gl2 = l1p.tile([128, 1024], f16, tag=f"gl2_{q}")
            btv = xl2b[(g, q)].rearrange("p (a b) -> p a b", b=34)
            for cchunk in range(2):
                psL2 = pbig.tile([128, 512], f32, tag="pbig")
                h0 = 16 * cchunk
                for dy in range(3):
                    nc.tensor.matmul(
                        psL2, csb["w2p2"][:, dy, :],
                        btv[:, h0 + dy:h0 + dy + 16, 0:32],
                        start=(dy == 0), stop=False)
                for dy in range(3):
                    nc.tensor.matmul(
                        psL2, csb["w2s2"][:, dy, :],
                        btv[0:64, h0 + dy:h0 + dy + 16, 2:34],
                        start=False, stop=(dy == 2))
                nc.scalar.activation(
                    out=gl2[:, 512 * cchunk:512 * cchunk + 512], in_=psL2,
                    func=AF.Gelu, bias=bn("cbn2b"), scale=bn("cbn2s"))

            # maxpool 32x32 -> 16x16 into the per-sample L3 band tiles
            pm2 = l1p.tile([128, 32, 16], f16, tag=f"pm2_{q}")
            w1v = gl2.rearrange("p (h w e) -> p h w e", w=16, e=2)
            nc.vector.tensor_tensor(out=pm2, in0=w1v[:, :, :, 0],
                                    in1=w1v[:, :, :, 1], op=ALU.max)
            w2v = pm2.rearrange("p (h e) w -> p h e w", e=2)
            for s2 in range(2):
                bt3 = xl3b[(g, q, s2)]
                nc.vector.tensor_tensor(
                    out=bt3[0:64].rearrange("p (a b) -> p a b", b=18)
                        [:, 1:17, 1:17],
                    in0=w2v[64 * s2:64 * s2 + 64, :, 0, :],
                    in1=w2v[64 * s2:64 * s2 + 64, :, 1, :], op=ALU.max)
                nc.vector.tensor_copy(out=bt3[64:128, 0:18 * 18 - 1],
                                      in_=bt3[0:64, 1:18 * 18])

        def emit_l3(g, q, s2):
            sg = 2 * q + s2
            bt3v = xl3b[(g, q, s2)].rearrange("p (a b) -> p a b", b=18)
            psL3 = pbig.tile([128, 256], f32, tag="pbig")
            for dy in range(3):
                nc.tensor.matmul(psL3, csb["w3p"][:, dy, :],
                                 bt3v[:, dy:dy + 16, 0:16],
                                 start=(dy == 0), stop=False)
            for dy in range(3):
                nc.tensor.matmul(psL3, csb["w3s"][:, dy, :],
                                 bt3v[0:64, dy:dy + 16, 2:18],
                                 start=False, stop=(dy == 2))
            gl3 = l1p.tile([128, 256], f16, tag=f"gl3_{s2}")
            nc.scalar.activation(out=gl3, in_=psL3, func=AF.Gelu,
                                 bias=bn("cbn3b"), scale=bn("cbn3s"))
            # maxpool 16x16 -> 8x8 into l4in (10x10 padded)
            pm3 = l1p.tile([128, 16, 8], f16, tag=f"pm3_{s2}")
            u1 = gl3.rearrange("p (h w e) -> p h w e", w=8, e=2)
            nc.vector.tensor_tensor(out=pm3, in0=u1[:, :, :, 0],
                                    in1=u1[:, :, :, 1], op=ALU.max)
            u2 = pm3.rearrange("p (h e) w -> p h e w", e=2)
            nc.vector.tensor_tensor(
                out=l4ins[g].rearrange("p (s a b) -> p s a b", a=10, b=10)
                    [:, sg, 1:9, 1:9],
                in0=u2[:, :, 0, :], in1=u2[:, :, 1, :], op=ALU.max)

        def emit_l4(g):
            psL4 = pbig.tile([128, 256], f32, tag="pbig")
            xl4 = l4ins[g].rearrange("p (s a b) -> p s a b", a=10, b=10)
            for t in range(9):
                dy, dx = t // 3, t % 3
                nc.tensor.matmul(psL4, csb["cw4T"][:, t, :],
                                 xl4[:, :, dy:dy + 8, dx:dx + 8],
                                 start=(t == 0), stop=(t == 8))
            gl4 = l1p.tile([128, 256], f16, tag="gl4")
            nc.scalar.activation(out=gl4, in_=psL4, func=AF.Gelu,
                                 bias=bn("cbn4b"), scale=bn("cbn4s"))
            # avgpool 8x8 -> 4x4 (sum; 0.25 folded into fc1 weights)
            av1 = l1p.tile([128, 128], f16, tag="av1")
            a1 = gl4.rearrange("p (s h w e) -> p s h w e", s=4, w=4, e=2)
            nc.vector.tensor_tensor(
                out=av1.rearrange("p (s h w) -> p s h w", s=4, w=4),
                in0=a1[:, :, :, :, 0], in1=a1[:, :, :, :, 1], op=ALU.add)
            a2 = av1.rearrange("p (s h e w) -> p s h e w", s=4, e=2, w=4)
            nc.vector.tensor_tensor(out=fcin[:, 64 * g:64 * g + 64]
                                        .rearrange("p (s h w) -> p s h w", s=4, w=4),
                                    in0=a2[:, :, :, 0, :], in1=a2[:, :, :, 1, :],
                                    op=ALU.add)

        emit_l2(0, 0)
        emit_l2(0, 1)
        emit_l3(0, 0, 0)
        emit_l3(0, 0, 1)
        emit_l3(0, 1, 0)
        emit_l3(0, 1, 1)
        emit_l2(1, 0)
        emit_l2(1, 1)
        emit_l4(0)
        emit_l3(1, 0, 0)
        emit_l3(1, 0, 1)
        emit_l3(1, 1, 0)
        emit_l3(1, 1, 1)
        emit_l4(1)

        # ================= FC head =================
        ps_fc = prp.tile([8, 256], f32, tag="prp")
        fv = fcin.rearrange("p (s j) -> p s j", j=16)
        for j in range(16):
            nc.tensor.matmul(ps_fc, fv[:, :, j], csb["fc1wT"][:, j, :],
                             start=(j == 0), stop=False)
        nc.tensor.matmul(ps_fc, onesK1M8, csb["fc1brow"], start=False, stop=True)
        nc.scalar.activation(out=fch, in_=ps_fc, func=AF.Gelu)
        junk = sing.tile([8, 256], f32)
        res8 = sing.tile([8, 1], f32)
        nc.vector.scalar_tensor_tensor(out=junk, in0=fch, scalar=1.0,
                                       in1=csb["fc2wb"], op0=ALU.mult,
                                       op1=ALU.mult, accum_out=res8)
        res8b = sing.tile([8, 1], f32)
        nc.vector.tensor_tensor(out=res8b, in0=res8, in1=csb["fc2bias"],
                                op=ALU.add)
        nc.sync.dma_start(out=out, in_=res8b)


# ------------------------------------------------------------------ driver
_prog_cache = {}


def _get_program(debug=False):
    key = ("dbg" if debug else "main")
    if key not in _prog_cache:
        _prog_cache[key] = build_program(debug=debug)
    return _prog_cache[key]


def _im2col_x(xs):
    """(8, 8, 512) f32 -> (112, 4, 512) f16 conv1d-1 im2col.

    Partition row 16k + 8s2 + c, pair p, col t = xpad[2p + s2, c, t + k]
    (pad 3 left/right).
    """
    xp = np.zeros((SPC, 8, T + 6), np.float16)
    xp[:, :, 3:3 + T] = xs.astype(np.float16)
    im = np.empty((7, 2, 8, 4, T), np.float16)
    for k in range(7):
        v = xp[:, :, k:k + T].reshape(4, 2, 8, T)
        im[k] = v.transpose(1, 2, 0, 3)
    return np.ascontiguousarray(im.reshape(112, 4, T))


def _run(inputs, debug=False):
    x = np.ascontiguousarray(np.asarray(inputs["x"]), np.float32)
    assert x.shape == (64, 8, 512), x.shape
    consts = _pack_consts({k: np.asarray(v) for k, v in inputs.items()})
    nc = _get_program(debug=debug)
    in_maps = []
    for c in range(N_CORES):
        m = dict(consts)
        m["xim"] = _im2col_x(x[SPC * c:SPC * c + SPC])
        in_maps.append(m)
    return run_bass_kernel_spmd(nc, in_maps, list(range(N_CORES)))


def kernel(**inputs):
    res = _run(inputs, debug=False)
    return np.concatenate([res.results[c]["out"][:, 0] for c in range(N_CORES)])
